# revision 1
# baseline (speedup 1.0000x reference)
"""Trainium2 Bass kernel for a pre-norm transformer encoder layer with RoPE,
causal attention and SwiGLU FFN.

Sharding: data-parallel over batch (B=8 -> 8 NeuronCores, one batch element
per core).  Each core runs the full layer on its [S=1300, D=1080] slice.

Per-core dataflow (feature-major activations for matmuls):
  P1  LN1 on token-major x, PE-transpose -> x2T (f32r)      [actT]
  P2  V = x2 @ Wv  (token-major), staged to DRAM scratch    [vscr]
  P3  per head: Q/K proj (M=90) + RoPE (rotation matmul), scoresT = K.Q^T,
      E = exp(scoresT/sqrt(dk)) with causal zero-mask (affine_select),
      attnT = V^T.E with ones-matmul denominator, normalize via gpsimd
      partition_broadcast, stage attnT to DRAM scratch      [ascr]
  P4  out-proj accumulated over heads, PE-transpose back to token-major,
      r1 = x + attn_out + bo                                [r1]
  P5  LN2 on r1, transpose -> x2'T (overwrites actT)
  P6/P7 (two 128-aligned S-chunks): H = silu(x2'@W1+b1)*(x2'@W3+b3),
      ffn_out = H^T.W2 + b2, transpose, accumulate-DMA into out

All matmuls run in float32r: full PE rate (1 cyc/row for N>=256) at ~1.6e-4
relative error.  Weights are passed from the host in pre-rearranged, padded
layouts so every weight DMA is a single fully-contiguous transfer.
"""

import sys

sys.path.insert(0, "/opt/trn_rl_repo")

import math

import numpy as np

B, S, D, H, DK, FF = 8, 1300, 1080, 12, 90, 3240
EPS = 1e-5

N_ST = (S + 127) // 128                      # 11 token tiles
SW = [128] * (N_ST - 1) + [S - 128 * (N_ST - 1)]   # last = 20
N_KT = (D + 127) // 128                      # 9
KP = [128] * (N_KT - 1) + [D - 128 * (N_KT - 1)]   # last = 56
QCH = [(0, 512), (512, 512), (1024, 276)]    # q chunks (128-aligned starts)
MT = 120
N_MT = D // MT                               # 9
N_FT = (FF + 127) // 128                     # 26
FSZ = [128] * (N_FT - 1) + [FF - 128 * (N_FT - 1)]  # last = 40
N_VB = 3
VBW = D // N_VB                              # 360
FH = [(0, 640), (640, 660)]                  # ffn S chunks (128-aligned)

_CACHE = {}


def _build():
    from contextlib import ExitStack

    import concourse.bacc as bacc
    import concourse.mybir as mybir
    import concourse.tile as tile

    f32 = mybir.dt.float32
    f32r = mybir.dt.float32r
    bf16 = mybir.dt.bfloat16
    AF = mybir.ActivationFunctionType
    OP = mybir.AluOpType

    nc = bacc.Bacc("TRN2", target_bir_lowering=False, debug=False)

    def din(name, shape, dt=f32):
        return nc.dram_tensor(name, shape, dt, kind="ExternalInput").ap()

    def dout(name, shape, dt=f32):
        return nc.dram_tensor(name, shape, dt, kind="ExternalOutput").ap()

    x_d = din("x", (S, D))
    wq_d = din("wqr", (H, 128, N_KT, DK), f32r)
    wk_d = din("wkr", (H, 128, N_KT, DK), f32r)
    wv_d = din("wvr", (128, N_KT, D), f32r)
    wo_d = din("wor", (N_MT, H, DK, MT), f32r)
    w1_d = din("w1r", (N_FT, 128, N_KT, 128), f32r)
    w3_d = din("w3r", (N_FT, 128, N_KT, 128), f32r)
    w2_d = din("w2r", (N_MT, 128, N_FT, MT), f32r)
    bq_d = din("bq", (D,))
    bk_d = din("bk", (D,))
    bv_d = din("bv", (D,))
    bo_d = din("bo", (D,))
    b1_d = din("b1", (FF,))
    b3_d = din("b3", (FF,))
    b2_d = din("b2", (D,))
    g1_d = din("g1", (D,))
    be1_d = din("be1", (D,))
    g2_d = din("g2", (D,))
    be2_d = din("be2", (D,))
    cost_d = din("cost", (DK, S))
    sint_d = din("sint", (DK, S))
    rl_d = din("rl", (DK, DK), f32r)
    ident_d = din("ident", (128, 128))
    cmask_d = din("cmask", (4, 128, 512))
    bvb_d = din("bvb", (128, D))
    onesrow_d = din("onesrow", (1, 128), f32r)

    out_d = dout("out", (S, D))
    vscr_d = dout("vscr", (H, 1408, DK), f32r)
    ascr_d = dout("ascr", (H, DK, S), f32r)

    SCALE = 1.0 / math.sqrt(DK)

    with tile.TileContext(nc) as tc, ExitStack() as ctx:
        glob = ctx.enter_context(tc.tile_pool(name="glob", bufs=1))
        work = ctx.enter_context(tc.tile_pool(name="work", bufs=2))
        psA = ctx.enter_context(tc.tile_pool(name="psA", bufs=2, space="PSUM"))
        psB = ctx.enter_context(tc.tile_pool(name="psB", bufs=2, space="PSUM"))
        psC = ctx.enter_context(tc.tile_pool(name="psC", bufs=2, space="PSUM"))
        psT = ctx.enter_context(tc.tile_pool(name="psT", bufs=2, space="PSUM"))

        # ---------- persistent tensors ----------
        actT = glob.tile([128, N_KT, S], f32r, tag="actT")    # x2T / x2'T
        pR1cm = tc.tile_pool(name="pR1", bufs=1)
        pR1 = pR1cm.__enter__()
        r1 = pR1.tile([128, N_ST, D], f32, tag="r1", name="r1")

        # ---------- constants ----------
        ident = glob.tile([128, 128], f32, tag="ident")
        nc.sync.dma_start(ident, ident_d)
        rl_s = glob.tile([DK, DK], f32r, tag="rl")
        nc.sync.dma_start(rl_s, rl_d)
        cosT = glob.tile([DK, S], f32, tag="cosT")
        nc.sync.dma_start(cosT, cost_d)
        sinT = glob.tile([DK, S], f32, tag="sinT")
        nc.sync.dma_start(sinT, sint_d)
        ones_t = glob.tile([128, 1], f32r, tag="ones")
        nc.vector.memset(ones_t.bitcast(f32), 1.0)
        eps_t = glob.tile([128, 1], f32, tag="eps")
        nc.vector.memset(eps_t, EPS)

        def col_param(name, dram, n, psz):
            """[N]-vector -> [128, ntiles] sbuf; tile i holds psz[i] rows."""
            t = glob.tile([128, n], f32, tag=name, name=name)
            full = sum(1 for p in psz if p == 128)
            if full:
                nc.sync.dma_start(
                    t[:, :full],
                    dram[0 : 128 * full].rearrange("(o p) -> p o", p=128),
                )
            for i in range(full, n):
                o = sum(psz[:i])
                nc.sync.dma_start(t[: psz[i], i : i + 1], dram[o : o + psz[i], None])
            return t

        g1_s = col_param("g1", g1_d, N_KT, KP)
        be1_s = col_param("be1", be1_d, N_KT, KP)
        g2_s = col_param("g2", g2_d, N_KT, KP)
        be2_s = col_param("be2", be2_d, N_KT, KP)
        b1_s = col_param("b1", b1_d, N_FT, FSZ)
        b3_s = col_param("b3", b3_d, N_FT, FSZ)
        bo_s = glob.tile([MT, N_MT], f32, tag="bo")
        nc.sync.dma_start(bo_s, bo_d.rearrange("(o p) -> p o", p=MT))
        b2_s = glob.tile([MT, N_MT], f32, tag="b2")
        nc.sync.dma_start(b2_s, b2_d.rearrange("(o p) -> p o", p=MT))
        bq_s = glob.tile([DK, H], f32, tag="bq")
        nc.sync.dma_start(bq_s, bq_d.rearrange("(o p) -> p o", p=DK))
        bk_s = glob.tile([DK, H], f32, tag="bk")
        nc.sync.dma_start(bk_s, bk_d.rearrange("(o p) -> p o", p=DK))
        bv_bc = glob.tile([128, D], f32, tag="bvbc")
        nc.sync.dma_start(bv_bc, bvb_d)
        cm_s = glob.tile([128, 4, 512], f32, tag="cmask")
        nc.sync.dma_start(cm_s, cmask_d.rearrange("t p f -> p t f"))
        ones_row = glob.tile([1, 128], f32r, tag="onesrow")
        nc.sync.dma_start(ones_row, onesrow_d)

        # ---------- helper: LN + transpose into actT ----------
        def layernorm_transpose(g_s, be_s):
            for st in range(N_ST):
                sw = SW[st]
                s0 = 128 * st
                xt = r1[:sw, st, :]
                ssum = work.tile([128, 1], f32, tag="ssum")
                nc.vector.reduce_sum(ssum[:sw], xt, axis=mybir.AxisListType.X)
                sqd = glob.tile([128, D], f32, tag="sqdump")
                ssq = work.tile([128, 1], f32, tag="ssq")
                nc.scalar.activation(sqd[:sw], xt, AF.Square, accum_out=ssq[:sw])
                mean = work.tile([128, 1], f32, tag="mean")
                nc.scalar.mul(mean[:sw], ssum[:sw], 1.0 / D)
                msq = work.tile([128, 1], f32, tag="msq")
                nc.vector.tensor_mul(msq[:sw], mean[:sw], mean[:sw])
                var = work.tile([128, 1], f32, tag="var")
                nc.vector.tensor_scalar_mul(var[:sw], ssq[:sw], 1.0 / D)
                nc.vector.tensor_sub(var[:sw], var[:sw], msq[:sw])
                std = work.tile([128, 1], f32, tag="std")
                nc.scalar.activation(std[:sw], var[:sw], AF.Sqrt, bias=eps_t[:sw])
                rstd = work.tile([128, 1], f32, tag="rstd")
                nc.vector.reciprocal(rstd[:sw], std[:sw])
                xn = work.tile([128, D], f32, tag="xn")
                nc.vector.tensor_scalar(
                    xn[:sw], xt, scalar1=mean[:sw], scalar2=rstd[:sw],
                    op0=OP.subtract, op1=OP.mult,
                )
                for kt in range(N_KT):
                    kp = KP[kt]
                    d0 = 128 * kt
                    pt = psT.tile([128, 128], f32, tag="pst")
                    nc.tensor.transpose(
                        pt[:kp, :sw], xn[:sw, d0 : d0 + kp], ident[:sw, :sw]
                    )
                    nc.scalar.activation(
                        actT[:kp, kt, s0 : s0 + sw],
                        pt[:kp, :sw],
                        AF.Identity,
                        bias=be_s[:kp, kt : kt + 1],
                        scale=g_s[:kp, kt : kt + 1],
                    )

        # ================= P1: LN1 =================
        for st in range(N_ST):
            nc.sync.dma_start(r1[: SW[st], st, :], x_d[128 * st : 128 * st + SW[st], :])
        layernorm_transpose(g1_s, be1_s)

        # ================= P2: V (token-major) -> vscr =================
        with tc.tile_pool(name="pP2", bufs=1) as pP2, \
             tc.tile_pool(name="pP2w", bufs=2) as pP2w:
            wv_t = pP2.tile([128, N_KT, D], f32r, tag="wv")
            for kt in range(N_KT):
                nc.sync.dma_start(wv_t[:, kt, :], wv_d[:, kt, :])
            for st in range(N_ST):
                sw = SW[st]
                s0 = 128 * st
                for vb in range(N_VB):
                    c0 = VBW * vb
                    pv = psA.tile([128, VBW], f32, tag="pa")
                    for kt in range(N_KT):
                        kp = KP[kt]
                        nc.tensor.matmul(
                            pv[:sw],
                            actT[:kp, kt, s0 : s0 + sw],
                            wv_t[:kp, kt, c0 : c0 + VBW],
                            start=(kt == 0),
                            stop=(kt == N_KT - 1),
                        )
                    vsb = pP2w.tile([128, VBW], f32r, tag="vsb")
                    nc.vector.tensor_tensor(
                        vsb[:sw], pv[:sw], bv_bc[:sw, c0 : c0 + VBW], OP.add
                    )
                    nc.sync.dma_start(
                        vscr_d[4 * vb : 4 * vb + 4, s0 : s0 + sw, :].rearrange(
                            "h s d -> s h d"
                        ),
                        vsb[:sw],
                    )

        # ================= P3: attention =================
        with tc.tile_pool(name="pP3", bufs=2) as pP3, \
             tc.tile_pool(name="pP3e", bufs=6) as pP3e:
            for h in range(H):
                c0 = DK * h
                wq_t = pP3.tile([128, N_KT, DK], f32r, tag="wq")
                nc.sync.dma_start(wq_t, wq_d[h])
                wk_t = pP3.tile([128, N_KT, DK], f32r, tag="wk")
                nc.sync.dma_start(wk_t, wk_d[h])

                qT = pP3.tile([DK, S], f32r, tag="qT")
                kT = pP3.tile([DK, S], f32r, tag="kT")
                for (w_t, b_s, outT) in ((wq_t, bq_s, qT), (wk_t, bk_s, kT)):
                    for (q0, qw) in QCH:
                        pq = psT.tile([DK, 512], f32, tag="pst")
                        for kt in range(N_KT):
                            kp = KP[kt]
                            nc.tensor.matmul(
                                pq[:, :qw],
                                w_t[:kp, kt, :],
                                actT[:kp, kt, q0 : q0 + qw],
                                start=(kt == 0),
                                stop=(kt == N_KT - 1),
                            )
                        raw = pP3.tile([DK, 512], f32r, tag="qraw")
                        nc.scalar.activation(
                            raw[:, :qw], pq[:, :qw], AF.Identity,
                            bias=b_s[:, h : h + 1],
                        )
                        prot = psT.tile([DK, 512], f32, tag="pst")
                        nc.tensor.matmul(
                            prot[:, :qw], rl_s, raw[:, :qw], start=True, stop=True
                        )
                        t1 = pP3.tile([DK, 512], f32, tag="ropet1")
                        nc.vector.tensor_tensor(
                            t1[:, :qw], raw[:, :qw].bitcast(f32),
                            cosT[:, q0 : q0 + qw], OP.mult,
                        )
                        t2 = pP3.tile([DK, 512], f32, tag="ropet2")
                        nc.vector.tensor_tensor(
                            t2[:, :qw], prot[:, :qw], sinT[:, q0 : q0 + qw], OP.mult
                        )
                        nc.vector.tensor_tensor(
                            outT[:, q0 : q0 + qw], t1[:, :qw], t2[:, :qw], OP.add
                        )

                vh = pP3.tile([128, N_ST, DK + 1], f32r, tag="vh")
                nc.sync.dma_start(
                    vh[:, :, :DK],
                    vscr_d[h].rearrange("(o p) d -> p o d", p=128),
                )

                for (q0, qw) in QCH:
                    kmax = min(N_ST, (q0 + qw + 127) // 128)
                    pat = psB.tile([DK, 512], f32, tag="pb")
                    pden = psC.tile([1, 512], f32, tag="pc")
                    for i in range(kmax):
                        ksz = SW[i]
                        pe = psA.tile([128, 512], f32, tag="pa")
                        nc.tensor.matmul(
                            pe[:ksz, :qw],
                            kT[:, 128 * i : 128 * i + ksz],
                            qT[:, q0 : q0 + qw],
                            start=True,
                            stop=True,
                        )
                        et = pP3e.tile([128, 512], f32r, tag="et")
                        nc.scalar.activation(
                            et[:ksz, :qw], pe[:ksz, :qw], AF.Exp, scale=SCALE
                        )
                        if 128 * i + ksz - 1 > q0:
                            t_ = i - q0 // 128
                            nc.vector.tensor_tensor(
                                et[:ksz, :qw], et[:ksz, :qw],
                                cm_s[:ksz, t_, :qw], OP.mult,
                            )
                        nc.tensor.matmul(
                            pat[:, :qw], vh[:ksz, i, :DK], et[:ksz, :qw],
                            start=(i == 0), stop=(i == kmax - 1),
                        )
                        nc.tensor.matmul(
                            pden[:, :qw], ones_t[:ksz], et[:ksz, :qw],
                            start=(i == 0), stop=(i == kmax - 1),
                        )
                    rec = pP3.tile([1, 512], f32r, tag="rec")
                    with nc.allow_low_precision(reason="f32r denom bcast"):
                        nc.vector.reciprocal(rec[:, :qw], pden[:, :qw])
                    bcp = psC.tile([DK, 512], f32, tag="pc")
                    nc.tensor.matmul(
                        bcp[:, :qw], ones_row[:1, :DK], rec[:, :qw],
                        start=True, stop=True,
                    )
                    bc = pP3.tile([DK, 512], f32, tag="bc")
                    nc.vector.tensor_copy(bc[:, :qw], bcp[:, :qw])
                    asb = pP3.tile([DK, 512], f32r, tag="asb")
                    nc.vector.tensor_tensor(
                        asb[:, :qw], pat[:, :qw], bc[:, :qw], OP.mult
                    )
                    nc.sync.dma_start(ascr_d[h, :, q0 : q0 + qw], asb[:, :qw])

        # ================= P4: out-proj + residual =================
        ACH = [(0, 640), (640, 660)]
        with tc.tile_pool(name="pP4", bufs=1) as pP4, \
             tc.tile_pool(name="pP4w", bufs=2) as pP4w, \
             tc.tile_pool(name="pP4o", bufs=2) as pP4o:
            for (q0, qw) in ACH:
                half = qw // 2
                sub = [(0, half), (half, qw - half)]
                arhs = [
                    pP4.tile([DK, 660], f32r, tag=f"ar{hh}", name=f"arhs{hh}")
                    for hh in range(H)
                ]
                for hh in range(H):
                    nc.sync.dma_start(arhs[hh][:, :qw], ascr_d[hh, :, q0 : q0 + qw])
                for mt in range(N_MT):
                    m0 = MT * mt
                    osb = pP4o.tile([MT, 660], f32, tag="osb")
                    wo_all = pP4w.tile([DK, H, MT], f32r, tag="woall")
                    nc.sync.dma_start(
                        wo_all, wo_d[mt].rearrange("h d m -> d h m")
                    )
                    for (so, sw_) in sub:
                        po = psA.tile([MT, 512], f32, tag="pa")
                        for hh in range(H):
                            nc.tensor.matmul(
                                po[:, :sw_], wo_all[:, hh, :],
                                arhs[hh][:, so : so + sw_],
                                start=(hh == 0), stop=(hh == H - 1),
                            )
                        nc.scalar.activation(
                            osb[:, so : so + sw_], po[:, :sw_], AF.Identity,
                            bias=bo_s[:, mt : mt + 1],
                        )
                    for j in range((qw + 127) // 128):
                        st = (q0 + 128 * j) // 128
                        sw = min(128, qw - 128 * j)
                        ptr = psT.tile([128, MT], f32, tag="pst")
                        nc.tensor.transpose(
                            ptr[:sw, :], osb[:, 128 * j : 128 * j + sw],
                            ident[:MT, :MT],
                        )
                        nc.vector.tensor_tensor(
                            r1[:sw, st, m0 : m0 + MT], r1[:sw, st, m0 : m0 + MT],
                            ptr[:sw, :], OP.add,
                        )

        # ================= P5: LN2 =================
        layernorm_transpose(g2_s, be2_s)

        # r1 is complete (LN2 consumed it): stage base of output to DRAM so
        # the r1 slab frees up for the FFN hidden tensor.
        for st in range(N_ST):
            nc.sync.dma_start(
                out_d[128 * st : 128 * st + SW[st], :], r1[: SW[st], st, :]
            )

        pR1cm.__exit__(None, None, None)

        # ================= P6/P7: FFN in two S chunks (all f32r) =================
        with tc.tile_pool(name="pF", bufs=1) as pF, \
             tc.tile_pool(name="pFw", bufs=2) as pFw, \
             tc.tile_pool(name="pFw2", bufs=1) as pFw2:
            for (hq0, hqw) in FH:
                Ht = pF.tile([128, N_FT, 660], f32r, tag="Ht", name="Ht")
                half = hqw // 2
                sub = [(0, half), (half, hqw - half)]
                for ft in range(N_FT):
                    fsz = FSZ[ft]
                    w1_t = pFw.tile([128, N_KT, 128], f32r, tag="w1")
                    nc.sync.dma_start(w1_t, w1_d[ft])
                    w3_t = pFw.tile([128, N_KT, 128], f32r, tag="w3")
                    nc.sync.dma_start(w3_t, w3_d[ft])
                    for (so, sw_) in sub:
                        g0 = hq0 + so
                        p1_ = psA.tile([128, 512], f32, tag="pa")
                        p3_ = psB.tile([128, 512], f32, tag="pb")
                        for kt in range(N_KT):
                            kp = KP[kt]
                            nc.tensor.matmul(
                                p1_[:fsz, :sw_], w1_t[:kp, kt, :fsz],
                                actT[:kp, kt, g0 : g0 + sw_],
                                start=(kt == 0), stop=(kt == N_KT - 1),
                            )
                            nc.tensor.matmul(
                                p3_[:fsz, :sw_], w3_t[:kp, kt, :fsz],
                                actT[:kp, kt, g0 : g0 + sw_],
                                start=(kt == 0), stop=(kt == N_KT - 1),
                            )
                        h1s = pFw.tile([128, 512], f32, tag="h1s")
                        nc.scalar.activation(
                            h1s[:fsz, :sw_], p1_[:fsz, :sw_], AF.Silu,
                            bias=b1_s[:fsz, ft : ft + 1],
                        )
                        h3b = pFw.tile([128, 512], f32, tag="h3b")
                        nc.scalar.activation(
                            h3b[:fsz, :sw_], p3_[:fsz, :sw_], AF.Identity,
                            bias=b3_s[:fsz, ft : ft + 1],
                        )
                        nc.vector.tensor_tensor(
                            Ht[:fsz, ft, so : so + sw_], h1s[:fsz, :sw_],
                            h3b[:fsz, :sw_], OP.mult,
                        )
                for mt in range(N_MT):
                    m0 = MT * mt
                    w2_t = pFw2.tile([128, N_FT, MT], f32r, tag="w2")
                    nc.sync.dma_start(w2_t, w2_d[mt])
                    fsb = pFw.tile([MT, 660], f32, tag="fsb")
                    for (so, sw_) in sub:
                        pf = psA.tile([MT, 512], f32, tag="pa")
                        for ft in range(N_FT):
                            fsz = FSZ[ft]
                            nc.tensor.matmul(
                                pf[:, :sw_], w2_t[:fsz, ft, :],
                                Ht[:fsz, ft, so : so + sw_],
                                start=(ft == 0), stop=(ft == N_FT - 1),
                            )
                        nc.scalar.activation(
                            fsb[:, so : so + sw_], pf[:, :sw_], AF.Identity,
                            bias=b2_s[:, mt : mt + 1],
                        )
                    for j in range((hqw + 127) // 128):
                        st = (hq0 + 128 * j) // 128
                        sw = min(128, hqw - 128 * j)
                        ptr = psT.tile([128, MT], f32, tag="pst")
                        nc.tensor.transpose(
                            ptr[:sw, :], fsb[:, 128 * j : 128 * j + sw],
                            ident[:MT, :MT],
                        )
                        stage = pFw.tile([128, MT], f32, tag="stage")
                        nc.vector.tensor_copy(stage[:sw], ptr[:sw])
                        nc.gpsimd.dma_start(
                            out_d[hq0 + 128 * j : hq0 + 128 * j + sw, m0 : m0 + MT],
                            stage[:sw],
                            accum_op=OP.add,
                        )

    nc.compile()
    return nc


def _host_inputs(inputs):
    """Shared (per-core-identical) input map pieces, from full inputs."""
    cos = np.ascontiguousarray(np.asarray(inputs["rope_cos"], np.float32).T)
    sin = np.ascontiguousarray(np.asarray(inputs["rope_sin"], np.float32).T)
    rl = np.zeros((DK, DK), np.float32)
    hdk = DK // 2
    rl[np.arange(hdk) + hdk, np.arange(hdk)] = -1.0
    rl[np.arange(hdk), np.arange(hdk) + hdk] = 1.0
    ident = np.eye(128, dtype=np.float32)
    f = lambda k: np.ascontiguousarray(np.asarray(inputs[k], np.float32))

    def pad_rows(w, rows):
        out = np.zeros((rows, w.shape[1]), np.float32)
        out[: w.shape[0]] = w
        return out

    Wq = f("Wq"); Wk = f("Wk"); Wv = f("Wv"); Wo = f("Wo")
    W1 = f("W1"); W3 = f("W3"); W2 = f("W2")
    KR = N_KT * 128
    # [H, 128, N_KT, DK]: (h, p, o, d) = Wq[o*128+p, h*90+d]
    wqr = np.ascontiguousarray(
        pad_rows(Wq, KR).reshape(N_KT, 128, H, DK).transpose(2, 1, 0, 3))
    wkr = np.ascontiguousarray(
        pad_rows(Wk, KR).reshape(N_KT, 128, H, DK).transpose(2, 1, 0, 3))
    # [128, N_KT, D]
    wvr = np.ascontiguousarray(pad_rows(Wv, KR).reshape(N_KT, 128, D).transpose(1, 0, 2))
    # [H, N_MT, DK, MT]
    wor = np.ascontiguousarray(Wo.reshape(H, DK, N_MT, MT).transpose(2, 0, 1, 3))
    # [N_FT, 128, N_KT, 128]: (ft, p, o, m) = W1[o*128+p, ft*128+m]
    FR = N_FT * 128
    w1p = np.zeros((KR, FR), np.float32); w1p[:D, :FF] = W1
    w3p = np.zeros((KR, FR), np.float32); w3p[:D, :FF] = W3
    w1r = np.ascontiguousarray(
        w1p.reshape(N_KT, 128, N_FT, 128).transpose(2, 1, 0, 3))
    w3r = np.ascontiguousarray(
        w3p.reshape(N_KT, 128, N_FT, 128).transpose(2, 1, 0, 3))
    # [N_MT, 128, N_FT, MT]: (mt, p, o, m) = W2[o*128+p, mt*120+m]
    w2p = np.zeros((FR, D), np.float32); w2p[:FF] = W2
    w2r = np.ascontiguousarray(
        w2p.reshape(N_FT, 128, N_MT, MT).transpose(2, 1, 0, 3))
    cmask = np.zeros((4, 128, 512), np.float32)
    for t in range(4):
        p_, f_ = np.mgrid[0:128, 0:512]
        cmask[t] = (f_ >= p_ + 128 * t).astype(np.float32)
    bvb = np.ascontiguousarray(
        np.broadcast_to(f("bv")[None, :], (128, D)).copy())
    onesrow = np.ones((1, 128), np.float32)
    return {
        "cmask": cmask, "bvb": bvb, "onesrow": onesrow,
        "wqr": wqr, "wkr": wkr, "wvr": wvr, "wor": wor,
        "w1r": w1r, "w3r": w3r, "w2r": w2r,
        "bq": f("bq"), "bk": f("bk"), "bv": f("bv"), "bo": f("bo"),
        "b1": f("b1"), "b3": f("b3"), "b2": f("b2"),
        "g1": f("ln1_g"), "be1": f("ln1_b"), "g2": f("ln2_g"), "be2": f("ln2_b"),
        "cost": cos, "sint": sin, "rl": rl, "ident": ident,
    }


def kernel(**inputs):
    from concourse.bass_utils import run_bass_kernel_spmd

    if "nc" not in _CACHE:
        _CACHE["nc"] = _build()
    nc = _CACHE["nc"]

    shared = _host_inputs(inputs)
    x = np.asarray(inputs["x"], np.float32)
    in_maps = [dict(shared, x=np.ascontiguousarray(x[b])) for b in range(B)]
    res = run_bass_kernel_spmd(nc, in_maps, list(range(B))).results
    out = np.stack([res[b]["out"] for b in range(B)], axis=0)
    return out.astype(np.float32)



# revision 9
# speedup vs baseline: 1.3922x; 1.3922x over previous
"""Trainium2 Bass kernel for a pre-norm transformer encoder layer with RoPE,
causal attention and SwiGLU FFN.

Sharding: data-parallel over batch (B=8 -> 8 NeuronCores, one batch element
per core).  Each core runs the full layer on its [S=1300, D=1080] slice.

v2 design (vs the DRAM-scratch baseline):
  - everything stays in SBUF: V, attn^T and the FFN hidden tensor are never
    staged to DRAM scratch.
  - all matmul operands are bf16 (psum fp32): halves SBUF footprint, halves
    weight DMA traffic, enables FWL weight loads and 4x DVE modes.
  - LN gamma/beta are folded into the projection weights host-side, so the
    LN transpose evacuation is a plain (batched) copy.
  - softmax denominator comes for free from a ones-column appended to V
    (attn^T matmul computes numerator rows 0..89 and the denominator row 90).
  - out-proj and FFN-W2 accumulate token-major straight into the fp32
    residual r1 (no PE transposes / accumulate-DMAs on the output path).

Per-core dataflow:
  P1  r1 = x (token-major, fp32); LN1 stats (bn_stats) -> xn (bf16),
      PE-transpose -> actT [128, 9, 1300] bf16; r1 += bo broadcast
  P2  V = x2 @ Wv per 360-col block -> v_sb [128, 11, 12, 91] bf16
      (col 90 memset to 1.0 = denominator ones-column)
  P3  per head: Q/K proj (psum) + bias, RoPE (rotation matmul + DVE),
      per q-chunk: scores K^T Q (psum pairs), exp (scalar, scale folded),
      causal mask on diagonal tiles, attnT = [V|1]^T E accumulated in psum,
      normalize via reciprocal + ones-row broadcast matmul -> aT bf16
  P4  out-proj token-major: po[s,360] = sum_h aT_h^T Wo_h; r1 += po
  P5  LN2 -> actT (overwrite); r1 += b2 broadcast
  P6  FFN in 3 S-chunks: Ht = silu(x2'W1+b1)*(x2'W3+b3) bf16;
      token-major W2: pf[s,360] = sum_ft Ht_ft^T W2_ft; r1 += pf
  P7  out = r1 (two DMAs)
"""

import sys

sys.path.insert(0, "/opt/trn_rl_repo")

import math

import numpy as np

B, S, D, H, DK, FF = 8, 1300, 1080, 12, 90, 3240
EPS = 1e-5

N_ST = (S + 127) // 128                      # 11 token tiles
SW = [128] * (N_ST - 1) + [S - 128 * (N_ST - 1)]   # last = 20
N_KT = (D + 127) // 128                      # 9
KP = [128] * (N_KT - 1) + [D - 128 * (N_KT - 1)]   # last = 56
QCH = [(0, 512), (512, 512), (1024, 276)]    # q/s chunks (128-aligned starts)
N_FT = (FF + 127) // 128                     # 26
FSZ = [128] * (N_FT - 1) + [FF - 128 * (N_FT - 1)]  # last = 40
N_VB = 3
VBW = D // N_VB                              # 360
TGRP = [(0, 4), (4, 4), (8, 1)]              # k-tile groups for LN evac

_CACHE = {}


def _build():
    from contextlib import ExitStack

    import concourse.bacc as bacc
    import concourse.mybir as mybir
    import concourse.tile as tile

    f32 = mybir.dt.float32
    bf16 = mybir.dt.bfloat16
    AF = mybir.ActivationFunctionType
    OP = mybir.AluOpType

    nc = bacc.Bacc("TRN2", target_bir_lowering=False, debug=False)

    def din(name, shape, dt=f32):
        return nc.dram_tensor(name, shape, dt, kind="ExternalInput").ap()

    def dout(name, shape, dt=f32):
        return nc.dram_tensor(name, shape, dt, kind="ExternalOutput").ap()

    x_d = din("x", (S, D))
    wq_d = din("wqr", (H, 128, N_KT, DK), bf16)
    wk_d = din("wkr", (H, 128, N_KT, DK), bf16)
    wv_d = din("wvr", (128, N_KT, D), bf16)
    wo_d = din("wor", (DK, H, D), bf16)
    w1_d = din("w1r", (N_FT, 128, N_KT, 128), bf16)
    w3_d = din("w3r", (N_FT, 128, N_KT, 128), bf16)
    w2_d = din("w2r", (128, N_FT, D), bf16)
    bq_d = din("bq", (DK, H))
    bk_d = din("bk", (DK, H))
    bvb_d = din("bvb", (128, D))
    bob_d = din("bob", (128, D))
    b2b_d = din("b2b", (128, D))
    b1_d = din("b1", (FF,))
    b3_d = din("b3", (FF,))
    cost_d = din("cost", (DK, S), bf16)
    sint_d = din("sint", (DK, S), bf16)
    rl_d = din("rl", (DK, DK), bf16)
    ident_d = din("ident", (128, 128), bf16)
    cmask_d = din("cmask", (4, 128, 512), bf16)
    onesrow_d = din("onesrow", (1, DK), bf16)

    out_d = dout("out", (S, D))

    SCALE = 1.0 / math.sqrt(DK)

    with tile.TileContext(nc) as tc, ExitStack() as ctx:
        glob = ctx.enter_context(tc.tile_pool(name="glob", bufs=1))
        work = ctx.enter_context(tc.tile_pool(name="work", bufs=3))
        psA = ctx.enter_context(tc.tile_pool(name="psA", bufs=2, space="PSUM"))
        psB = ctx.enter_context(tc.tile_pool(name="psB", bufs=2, space="PSUM"))
        psT = ctx.enter_context(tc.tile_pool(name="psT", bufs=2, space="PSUM"))

        # ---------- persistent tensors ----------
        actT = glob.tile([128, N_KT, S], bf16, tag="actT")    # x2T / x2'T
        r1 = glob.tile([128, N_ST, D], f32, tag="r1")         # residual

        # ---------- persistent constants ----------
        ident = glob.tile([128, 128], bf16, tag="ident")
        nc.sync.dma_start(ident, ident_d)
        eps_t = glob.tile([128, 1], f32, tag="eps")
        nc.vector.memset(eps_t, EPS)
        bo_bc = glob.tile([128, D], f32, tag="bobc")
        nc.sync.dma_start(bo_bc, bob_d)
        b2_bc = glob.tile([128, D], f32, tag="b2bc")
        nc.sync.dma_start(b2_bc, b2b_d)

        def col_param(pool, name, dram, n, psz):
            t = pool.tile([128, n], f32, tag=name, name=name)
            full = sum(1 for p in psz if p == 128)
            if full:
                nc.sync.dma_start(
                    t[:, :full],
                    dram[0 : 128 * full].rearrange("(o p) -> p o", p=128),
                )
            for i in range(full, n):
                o = sum(psz[:i])
                nc.sync.dma_start(t[: psz[i], i : i + 1], dram[o : o + psz[i], None])
            return t

        # ---------- load x into r1 (token-major) ----------
        nc.sync.dma_start(
            r1[:, : N_ST - 1, :],
            x_d[: 128 * (N_ST - 1), :].rearrange("(o p) d -> p o d", p=128),
        )
        nc.sync.dma_start(r1[: SW[-1], N_ST - 1, :], x_d[128 * (N_ST - 1) :, :])

        # ---------- helper: LN + transpose into actT (gamma/beta folded into
        # the downstream weights, so the evacuation is a plain copy) ----------
        def layernorm_transpose():
            for st in range(N_ST):
                sw = SW[st]
                s0 = 128 * st
                xt = r1[:sw, st, :]
                stats = work.tile([128, 3, 6], f32, tag="stats")
                for j in range(3):
                    nc.vector.bn_stats(
                        stats[:sw, j, :], xt[:, VBW * j : VBW * (j + 1)]
                    )
                mv = work.tile([128, 2], f32, tag="mv")
                nc.vector.bn_aggr(mv[:sw], stats[:sw])
                std = work.tile([128, 1], f32, tag="std")
                nc.scalar.activation(
                    std[:sw], mv[:sw, 1:2], AF.Sqrt, bias=eps_t[:sw]
                )
                rstd = work.tile([128, 1], f32, tag="rstd")
                nc.vector.reciprocal(rstd[:sw], std[:sw])
                xn = work.tile([128, D], bf16, tag="xn")
                nc.vector.tensor_scalar(
                    xn[:sw], xt, scalar1=mv[:sw, 0:1], scalar2=rstd[:sw],
                    op0=OP.subtract, op1=OP.mult,
                )
                for (k0, kn) in TGRP:
                    pt = psT.tile([128, 512], bf16, tag="pst")
                    for kt in range(k0, k0 + kn):
                        kp = KP[kt]
                        nc.tensor.transpose(
                            pt[:kp, 128 * (kt - k0) : 128 * (kt - k0) + sw],
                            xn[:sw, 128 * kt : 128 * kt + kp],
                            ident[:sw, :sw],
                        )
                    kpg = KP[k0]  # 128 for full groups, 56 for the tail
                    if kn == 1 or sw == 128:
                        nc.scalar.activation(
                            actT[:kpg, k0 : k0 + kn, s0 : s0 + sw],
                            pt[:kpg, : 128 * (kn - 1) + sw],
                            AF.Identity,
                        )
                    else:
                        for kt in range(k0, k0 + kn):
                            nc.scalar.activation(
                                actT[: KP[kt], kt, s0 : s0 + sw],
                                pt[: KP[kt], 128 * (kt - k0) : 128 * (kt - k0) + sw],
                                AF.Identity,
                            )

        # ================= P1: LN1 =================
        layernorm_transpose()
        for st in range(N_ST):
            nc.vector.tensor_tensor(
                r1[: SW[st], st, :], r1[: SW[st], st, :], bo_bc[: SW[st]], OP.add
            )

        # ============ P2 + P3 + P4 (attention, scoped pools) ============
        pVcm = tc.tile_pool(name="pV", bufs=1)
        pV = pVcm.__enter__()
        v_sb = pV.tile([128, N_ST, H, DK + 7], bf16, tag="vsb", name="v_sb")
        nc.vector.memset(v_sb, 1.0)

        # ---- P2: V (token-major, +ones col) ----
        with tc.tile_pool(name="pP2w", bufs=1) as pP2w:
            wv_t = pP2w.tile([128, N_KT, D], bf16, tag="wv")
            for kt in range(N_KT):
                nc.sync.dma_start(wv_t[:, kt, :], wv_d[:, kt, :])
            bv_bc = pP2w.tile([128, D], f32, tag="bvbc")
            nc.sync.dma_start(bv_bc, bvb_d)
            for vb in range(N_VB):
                c0 = VBW * vb
                for st in range(N_ST):
                    sw = SW[st]
                    pv = psA.tile([128, 2, 512], f32, tag="pa")
                    for kt in range(N_KT):
                        kp = KP[kt]
                        nc.tensor.matmul(
                            pv[:sw, 0, :VBW],
                            actT[:kp, kt, 128 * st : 128 * st + sw],
                            wv_t[:kp, kt, c0 : c0 + VBW],
                            start=(kt == 0),
                            stop=(kt == N_KT - 1),
                        )
                    for hh in range(4):
                        h = 4 * vb + hh
                        nc.vector.tensor_tensor(
                            v_sb[:sw, st, h, :DK],
                            pv[:sw, 0, DK * hh : DK * hh + DK],
                            bv_bc[:sw, DK * h : DK * h + DK],
                            OP.add,
                        )

        # ---- P3: attention ----
        pATcm = tc.tile_pool(name="pAT", bufs=1)
        pAT = pATcm.__enter__()
        aT = pAT.tile([DK, H, S], bf16, tag="aT", name="aT")
        with tc.tile_pool(name="pP3c", bufs=1) as pP3c, \
             tc.tile_pool(name="pP3", bufs=2) as pP3, \
             tc.tile_pool(name="pP3e", bufs=4) as pP3e:
            rl_s = pP3c.tile([DK, DK], bf16, tag="rl")
            nc.sync.dma_start(rl_s, rl_d)
            cosT = pP3c.tile([DK, S], bf16, tag="cosT")
            nc.sync.dma_start(cosT, cost_d)
            sinT = pP3c.tile([DK, S], bf16, tag="sinT")
            nc.sync.dma_start(sinT, sint_d)
            cm_s = pP3c.tile([128, 4, 512], bf16, tag="cmask")
            nc.sync.dma_start(cm_s, cmask_d.rearrange("t p f -> p t f"))
            ones_row = pP3c.tile([1, DK], bf16, tag="onesrow")
            nc.sync.dma_start(ones_row, onesrow_d)
            bq_s = pP3c.tile([DK, H], f32, tag="bq")
            nc.sync.dma_start(bq_s, bq_d)
            bk_s = pP3c.tile([DK, H], f32, tag="bk")
            nc.sync.dma_start(bk_s, bk_d)

            for h in range(H):
                wq_t = pP3.tile([128, N_KT, DK], bf16, tag="wq")
                nc.sync.dma_start(wq_t, wq_d[h])
                wk_t = pP3.tile([128, N_KT, DK], bf16, tag="wk")
                nc.sync.dma_start(wk_t, wk_d[h])

                qT = pP3.tile([DK, S], bf16, tag="qT")
                kT = pP3.tile([DK, S], bf16, tag="kT")
                for (w_t, b_s, outT) in ((wq_t, bq_s, qT), (wk_t, bk_s, kT)):
                    for (q0, qw) in QCH:
                        pq = psT.tile([128, 512], f32, tag="pst")
                        for kt in range(N_KT):
                            kp = KP[kt]
                            nc.tensor.matmul(
                                pq[:DK, :qw],
                                w_t[:kp, kt, :],
                                actT[:kp, kt, q0 : q0 + qw],
                                start=(kt == 0),
                                stop=(kt == N_KT - 1),
                            )
                        raw = pP3.tile([DK, 512], bf16, tag="qraw")
                        nc.scalar.activation(
                            raw[:, :qw], pq[:DK, :qw], AF.Identity,
                            bias=b_s[:, h : h + 1],
                        )
                        prot = psT.tile([128, 512], f32, tag="pst")
                        nc.tensor.matmul(
                            prot[:DK, :qw], rl_s, raw[:, :qw], start=True, stop=True
                        )
                        t1 = pP3.tile([DK, 512], bf16, tag="ropet1")
                        nc.vector.tensor_tensor(
                            t1[:, :qw], raw[:, :qw], cosT[:, q0 : q0 + qw], OP.mult
                        )
                        t2 = pP3.tile([DK, 512], bf16, tag="ropet2")
                        nc.vector.tensor_tensor(
                            t2[:, :qw], prot[:DK, :qw], sinT[:, q0 : q0 + qw],
                            OP.mult,
                        )
                        nc.vector.tensor_tensor(
                            outT[:, q0 : q0 + qw], t1[:, :qw], t2[:, :qw], OP.add
                        )

                for (q0, qw) in QCH:
                    kmax = min(N_ST, (q0 + qw + 127) // 128)
                    pat = psB.tile([128, 512], f32, tag="pb")
                    for i0 in range(0, kmax, 2):
                        kn = min(2, kmax - i0)
                        pe2 = psA.tile([128, 2, 512], f32, tag="pa")
                        for j in range(kn):
                            i = i0 + j
                            ksz = SW[i]
                            nc.tensor.matmul(
                                pe2[:ksz, j, :qw],
                                kT[:, 128 * i : 128 * i + ksz],
                                qT[:, q0 : q0 + qw],
                                start=True,
                                stop=True,
                            )
                        et = pP3e.tile([128, 2, 512], bf16, tag="et")
                        ks0 = SW[i0]
                        if kn == 2 and SW[i0 + 1] == ks0:
                            nc.scalar.activation(
                                et[:ks0, :, :qw], pe2[:ks0, :, :qw], AF.Exp,
                                scale=SCALE,
                            )
                        else:
                            for j in range(kn):
                                nc.scalar.activation(
                                    et[: SW[i0 + j], j, :qw],
                                    pe2[: SW[i0 + j], j, :qw],
                                    AF.Exp,
                                    scale=SCALE,
                                )
                        for j in range(kn):
                            i = i0 + j
                            ksz = SW[i]
                            if 128 * i + ksz - 1 > q0:
                                t_ = i - q0 // 128
                                nc.vector.tensor_tensor(
                                    et[:ksz, j, :qw], et[:ksz, j, :qw],
                                    cm_s[:ksz, t_, :qw], OP.mult,
                                )
                            nc.tensor.matmul(
                                pat[: DK + 7, :qw],
                                v_sb[:ksz, i, h, :],
                                et[:ksz, j, :qw],
                                start=(i == 0),
                                stop=(i == kmax - 1),
                            )
                    rec = pP3.tile([1, 512], bf16, tag="rec")
                    with nc.allow_low_precision(reason="bf16 denom bcast"):
                        nc.vector.reciprocal(rec[:, :qw], pat[DK + 6 : DK + 7, :qw])
                    bcp = psT.tile([128, 512], f32, tag="pst")
                    nc.tensor.matmul(
                        bcp[:DK, :qw], ones_row, rec[:, :qw], start=True, stop=True
                    )
                    bc = pP3.tile([DK, 512], bf16, tag="bc")
                    nc.scalar.activation(bc[:, :qw], bcp[:DK, :qw], AF.Identity)
                    nc.vector.tensor_tensor(
                        aT[:, h, q0 : q0 + qw], pat[:DK, :qw], bc[:, :qw], OP.mult
                    )

        # ---- P4: out-proj token-major + residual ----
        with tc.tile_pool(name="pP4", bufs=1) as pP4:
            wo_s = pP4.tile([DK, H, D], bf16, tag="wo")
            nc.sync.dma_start(wo_s, wo_d)
            for st in range(N_ST):
                sw = SW[st]
                s0 = 128 * st
                for vb in range(N_VB):
                    c0 = VBW * vb
                    po = psA.tile([128, 2, 512], f32, tag="pa")
                    for h in range(H):
                        nc.tensor.matmul(
                            po[:sw, 0, :VBW],
                            aT[:, h, s0 : s0 + sw],
                            wo_s[:, h, c0 : c0 + VBW],
                            start=(h == 0),
                            stop=(h == H - 1),
                        )
                    nc.vector.tensor_tensor(
                        r1[:sw, st, c0 : c0 + VBW], r1[:sw, st, c0 : c0 + VBW],
                        po[:sw, 0, :VBW], OP.add,
                    )

        pATcm.__exit__(None, None, None)
        pVcm.__exit__(None, None, None)

        # ================= P5: LN2 =================
        layernorm_transpose()
        for st in range(N_ST):
            nc.vector.tensor_tensor(
                r1[: SW[st], st, :], r1[: SW[st], st, :], b2_bc[: SW[st]], OP.add
            )

        # ================= P6: FFN in 3 S-chunks ==========================
        with tc.tile_pool(name="pW2", bufs=1) as pW2, \
             tc.tile_pool(name="pF", bufs=2) as pF, \
             tc.tile_pool(name="pFh", bufs=1) as pFh:
            w2_s = pW2.tile([128, N_FT, D], bf16, tag="w2")
            for ft in range(N_FT):
                nc.sync.dma_start(w2_s[:, ft, :], w2_d[:, ft, :])
            b1_s = col_param(pW2, "b1", b1_d, N_FT, FSZ)
            b3_s = col_param(pW2, "b3", b3_d, N_FT, FSZ)
            Ht = pFh.tile([128, N_FT, 512], bf16, tag="Ht", name="Ht")
            for (g0, gw) in QCH:
                for ft in range(N_FT):
                    fsz = FSZ[ft]
                    w1_t = pF.tile([128, N_KT, 128], bf16, tag="w1")
                    nc.sync.dma_start(w1_t, w1_d[ft])
                    w3_t = pF.tile([128, N_KT, 128], bf16, tag="w3")
                    nc.sync.dma_start(w3_t, w3_d[ft])
                    p1_ = psA.tile([128, 2, 512], f32, tag="pa")
                    p3_ = psB.tile([128, 512], f32, tag="pb")
                    for kt in range(N_KT):
                        kp = KP[kt]
                        nc.tensor.matmul(
                            p1_[:fsz, 0, :gw], w1_t[:kp, kt, :fsz],
                            actT[:kp, kt, g0 : g0 + gw],
                            start=(kt == 0), stop=(kt == N_KT - 1),
                        )
                        nc.tensor.matmul(
                            p3_[:fsz, :gw], w3_t[:kp, kt, :fsz],
                            actT[:kp, kt, g0 : g0 + gw],
                            start=(kt == 0), stop=(kt == N_KT - 1),
                        )
                    h1s = pF.tile([128, 512], bf16, tag="h1s")
                    nc.scalar.activation(
                        h1s[:fsz, :gw], p1_[:fsz, 0, :gw], AF.Silu,
                        bias=b1_s[:fsz, ft : ft + 1],
                    )
                    h3b = pF.tile([128, 512], bf16, tag="h3b")
                    nc.scalar.activation(
                        h3b[:fsz, :gw], p3_[:fsz, :gw], AF.Identity,
                        bias=b3_s[:fsz, ft : ft + 1],
                    )
                    nc.vector.tensor_tensor(
                        Ht[:fsz, ft, :gw], h1s[:fsz, :gw], h3b[:fsz, :gw], OP.mult
                    )
                for j in range((gw + 127) // 128):
                    st = g0 // 128 + j
                    sw = SW[st]
                    for vb in range(N_VB):
                        c0 = VBW * vb
                        pf = psA.tile([128, 2, 512], f32, tag="pa")
                        for ft in range(N_FT):
                            fsz = FSZ[ft]
                            nc.tensor.matmul(
                                pf[:sw, 0, :VBW],
                                Ht[:fsz, ft, 128 * j : 128 * j + sw],
                                w2_s[:fsz, ft, c0 : c0 + VBW],
                                start=(ft == 0), stop=(ft == N_FT - 1),
                            )
                        nc.vector.tensor_tensor(
                            r1[:sw, st, c0 : c0 + VBW],
                            r1[:sw, st, c0 : c0 + VBW],
                            pf[:sw, 0, :VBW], OP.add,
                        )

        # ================= P7: store =================
        nc.sync.dma_start(
            out_d[: 128 * (N_ST - 1), :].rearrange("(o p) d -> p o d", p=128),
            r1[:, : N_ST - 1, :],
        )
        nc.sync.dma_start(out_d[128 * (N_ST - 1) :, :], r1[: SW[-1], N_ST - 1, :])

    nc.compile()
    return nc


def _host_inputs(inputs):
    """Shared (per-core-identical) input map pieces, from full inputs."""
    import ml_dtypes

    f = lambda k: np.asarray(inputs[k], np.float32)

    def to_bf16(a):
        return np.ascontiguousarray(np.asarray(a, np.float32)).astype(
            ml_dtypes.bfloat16
        )

    g1 = f("ln1_g"); be1 = f("ln1_b"); g2 = f("ln2_g"); be2 = f("ln2_b")
    Wq = f("Wq") * g1[:, None]
    Wk = f("Wk") * g1[:, None]
    Wv = f("Wv") * g1[:, None]
    Wo = f("Wo")
    W1 = f("W1") * g2[:, None]
    W3 = f("W3") * g2[:, None]
    W2 = f("W2")
    bq = f("bq") + be1 @ f("Wq")
    bk = f("bk") + be1 @ f("Wk")
    bv = f("bv") + be1 @ f("Wv")
    b1 = f("b1") + be2 @ f("W1")
    b3 = f("b3") + be2 @ f("W3")
    bo = f("bo")
    b2 = f("b2")

    cos = np.ascontiguousarray(f("rope_cos").T)   # [DK, S]
    sin = np.ascontiguousarray(f("rope_sin").T)
    rl = np.zeros((DK, DK), np.float32)
    hdk = DK // 2
    rl[np.arange(hdk) + hdk, np.arange(hdk)] = -1.0
    rl[np.arange(hdk), np.arange(hdk) + hdk] = 1.0
    ident = np.eye(128, dtype=np.float32)

    def pad_rows(w, rows):
        out = np.zeros((rows, w.shape[1]), np.float32)
        out[: w.shape[0]] = w
        return out

    KR = N_KT * 128
    # [H, 128, N_KT, DK]: (h, p, o, d) = Wq[o*128+p, h*90+d]
    wqr = pad_rows(Wq, KR).reshape(N_KT, 128, H, DK).transpose(2, 1, 0, 3)
    wkr = pad_rows(Wk, KR).reshape(N_KT, 128, H, DK).transpose(2, 1, 0, 3)
    # [128, N_KT, D]
    wvr = pad_rows(Wv, KR).reshape(N_KT, 128, D).transpose(1, 0, 2)
    # [DK, H, D]: (p, h, c) = Wo[h*90+p, c]
    wor = Wo.reshape(H, DK, D).transpose(1, 0, 2)
    # [N_FT, 128, N_KT, 128]: (ft, p, o, m) = W1[o*128+p, ft*128+m]
    FR = N_FT * 128
    w1p = np.zeros((KR, FR), np.float32); w1p[:D, :FF] = W1
    w3p = np.zeros((KR, FR), np.float32); w3p[:D, :FF] = W3
    w1r = w1p.reshape(N_KT, 128, N_FT, 128).transpose(2, 1, 0, 3)
    w3r = w3p.reshape(N_KT, 128, N_FT, 128).transpose(2, 1, 0, 3)
    # [128, N_FT, D]: (p, ft, c) = W2[ft*128+p, c]
    w2p = np.zeros((FR, D), np.float32); w2p[:FF] = W2
    w2r = w2p.reshape(N_FT, 128, D).transpose(1, 0, 2)

    cmask = np.zeros((4, 128, 512), np.float32)
    for t in range(4):
        p_, f_ = np.mgrid[0:128, 0:512]
        cmask[t] = (f_ >= p_ + 128 * t).astype(np.float32)
    bvb = np.broadcast_to(bv[None, :], (128, D)).copy()
    bob = np.broadcast_to(bo[None, :], (128, D)).copy()
    b2b = np.broadcast_to(b2[None, :], (128, D)).copy()
    onesrow = np.ones((1, DK), np.float32)
    return {
        "wqr": to_bf16(wqr), "wkr": to_bf16(wkr), "wvr": to_bf16(wvr),
        "wor": to_bf16(wor), "w1r": to_bf16(w1r), "w3r": to_bf16(w3r),
        "w2r": to_bf16(w2r),
        "bq": np.ascontiguousarray(bq.reshape(H, DK).T),
        "bk": np.ascontiguousarray(bk.reshape(H, DK).T),
        "bvb": bvb, "bob": bob, "b2b": b2b,
        "b1": b1, "b3": b3,
        "cost": to_bf16(cos), "sint": to_bf16(sin), "rl": to_bf16(rl),
        "ident": to_bf16(ident), "cmask": to_bf16(cmask),
        "onesrow": to_bf16(onesrow),
    }


def kernel(**inputs):
    from concourse.bass_utils import run_bass_kernel_spmd

    if "nc" not in _CACHE:
        _CACHE["nc"] = _build()
    nc = _CACHE["nc"]

    shared = _host_inputs(inputs)
    x = np.asarray(inputs["x"], np.float32)
    in_maps = [dict(shared, x=np.ascontiguousarray(x[b])) for b in range(B)]
    res = run_bass_kernel_spmd(nc, in_maps, list(range(B))).results
    out = np.stack([res[b]["out"] for b in range(B)], axis=0)
    return out.astype(np.float32)


# revision 10
# speedup vs baseline: 1.7472x; 1.2550x over previous
"""Trainium2 Bass kernel for a pre-norm transformer encoder layer with RoPE,
causal attention and SwiGLU FFN.

Sharding: data-parallel over batch (B=8 -> 8 NeuronCores, one batch element
per core).  Each core runs the full layer on its [S=1300, D=1080] slice.

v2 design (vs the DRAM-scratch baseline):
  - everything stays in SBUF: V, attn^T and the FFN hidden tensor are never
    staged to DRAM scratch.
  - all matmul operands are bf16 (psum fp32): halves SBUF footprint, halves
    weight DMA traffic, enables FWL weight loads and 4x DVE modes.
  - LN gamma/beta are folded into the projection weights host-side, so the
    LN transpose evacuation is a plain (batched) copy.
  - softmax denominator comes for free from a ones-column appended to V
    (attn^T matmul computes numerator rows 0..89 and the denominator row 90).
  - out-proj and FFN-W2 accumulate token-major straight into the fp32
    residual r1 (no PE transposes / accumulate-DMAs on the output path).

Per-core dataflow:
  P1  r1 = x (token-major, fp32); LN1 stats (bn_stats) -> xn (bf16),
      PE-transpose -> actT [128, 9, 1300] bf16; r1 += bo broadcast
  P2  V = x2 @ Wv per 360-col block -> v_sb [128, 11, 12, 91] bf16
      (col 90 memset to 1.0 = denominator ones-column)
  P3  per head: Q/K proj (psum) + bias, RoPE (rotation matmul + DVE),
      per q-chunk: scores K^T Q (psum pairs), exp (scalar, scale folded),
      causal mask on diagonal tiles, attnT = [V|1]^T E accumulated in psum,
      normalize via reciprocal + ones-row broadcast matmul -> aT bf16
  P4  out-proj token-major: po[s,360] = sum_h aT_h^T Wo_h; r1 += po
  P5  LN2 -> actT (overwrite); r1 += b2 broadcast
  P6  FFN in 3 S-chunks: Ht = silu(x2'W1+b1)*(x2'W3+b3) bf16;
      token-major W2: pf[s,360] = sum_ft Ht_ft^T W2_ft; r1 += pf
  P7  out = r1 (two DMAs)
"""

import sys

sys.path.insert(0, "/opt/trn_rl_repo")

import math

import numpy as np

B, S, D, H, DK, FF = 8, 1300, 1080, 12, 90, 3240
EPS = 1e-5

N_ST = (S + 127) // 128                      # 11 token tiles
SW = [128] * (N_ST - 1) + [S - 128 * (N_ST - 1)]   # last = 20
N_KT = (D + 127) // 128                      # 9
KP = [128] * (N_KT - 1) + [D - 128 * (N_KT - 1)]   # last = 56
QCH = [(0, 512), (512, 512), (1024, 276)]    # q/s chunks (128-aligned starts)
N_FT = (FF + 127) // 128                     # 26
FSZ = [128] * (N_FT - 1) + [FF - 128 * (N_FT - 1)]  # last = 40
N_VB = 3
VBW = D // N_VB                              # 360
TGRP = [(0, 4), (4, 4), (8, 1)]              # k-tile groups for LN evac

_CACHE = {}


def _build():
    from contextlib import ExitStack

    import concourse.bacc as bacc
    import concourse.mybir as mybir
    import concourse.tile as tile

    f32 = mybir.dt.float32
    bf16 = mybir.dt.bfloat16
    AF = mybir.ActivationFunctionType
    OP = mybir.AluOpType

    nc = bacc.Bacc("TRN2", target_bir_lowering=False, debug=False)

    def din(name, shape, dt=f32):
        return nc.dram_tensor(name, shape, dt, kind="ExternalInput").ap()

    def dout(name, shape, dt=f32):
        return nc.dram_tensor(name, shape, dt, kind="ExternalOutput").ap()

    x_d = din("x", (S, D))
    wqk_d = din("wqkr", (H, 128, N_KT, 2, DK), bf16)
    wv_d = din("wvr", (128, N_KT, D), bf16)
    wo_d = din("wor", (DK, H, D), bf16)
    w13_d = din("w13r", (N_FT, 128, N_KT, 2, 128), bf16)
    w2_d = din("w2r", (128, N_FT, D), bf16)
    bqk_d = din("bqk", (DK, 2, H))
    ball_d = din("ball", (128, 3, D))
    b1_d = din("b1", (FF,))
    b3_d = din("b3", (FF,))
    cost_d = din("cost", (DK, S), bf16)
    sint_d = din("sint", (DK, S), bf16)
    rl_d = din("rl", (DK, DK), bf16)
    ident_d = din("ident", (128, 128), bf16)
    cmask_d = din("cmask", (4, 128, 512), bf16)

    out_d = dout("out", (S, D))

    SCALE = 1.0 / math.sqrt(DK)

    with tile.TileContext(nc) as tc, ExitStack() as ctx:
        glob = ctx.enter_context(tc.tile_pool(name="glob", bufs=1))
        work = ctx.enter_context(tc.tile_pool(name="work", bufs=3))
        psA = ctx.enter_context(tc.tile_pool(name="psA", bufs=2, space="PSUM"))
        psB = ctx.enter_context(tc.tile_pool(name="psB", bufs=2, space="PSUM"))
        psT = ctx.enter_context(tc.tile_pool(name="psT", bufs=2, space="PSUM"))

        # ---------- persistent tensors ----------
        actT = glob.tile([128, N_KT, S], bf16, tag="actT")    # x2T / x2'T
        r1 = glob.tile([128, N_ST, D], f32, tag="r1")         # residual

        # ---------- persistent constants ----------
        ident = glob.tile([128, 128], bf16, tag="ident")
        nc.sync.dma_start(ident, ident_d)
        eps_t = glob.tile([128, 1], f32, tag="eps")
        nc.vector.memset(eps_t, EPS)
        ball = glob.tile([128, 3, D], f32, tag="ball")
        nc.sync.dma_start(ball, ball_d)
        bv_bc = ball[:, 0, :]
        bo_bc = ball[:, 1, :]
        b2_bc = ball[:, 2, :]

        def col_param(pool, name, dram, n, psz):
            t = pool.tile([128, n], f32, tag=name, name=name)
            full = sum(1 for p in psz if p == 128)
            if full:
                nc.sync.dma_start(
                    t[:, :full],
                    dram[0 : 128 * full].rearrange("(o p) -> p o", p=128),
                )
            for i in range(full, n):
                o = sum(psz[:i])
                nc.sync.dma_start(t[: psz[i], i : i + 1], dram[o : o + psz[i], None])
            return t

        # ---------- load x into r1 (token-major) ----------
        nc.sync.dma_start(
            r1[:, : N_ST - 1, :],
            x_d[: 128 * (N_ST - 1), :].rearrange("(o p) d -> p o d", p=128),
        )
        nc.sync.dma_start(r1[: SW[-1], N_ST - 1, :], x_d[128 * (N_ST - 1) :, :])

        # ---------- helper: LN + transpose into actT (gamma/beta folded into
        # the downstream weights, so the evacuation is a plain copy) ----------
        def layernorm_transpose():
            for st in range(N_ST):
                sw = SW[st]
                s0 = 128 * st
                xt = r1[:sw, st, :]
                stats = work.tile([128, 3, 6], f32, tag="stats")
                for j in range(3):
                    nc.vector.bn_stats(
                        stats[:sw, j, :], xt[:, VBW * j : VBW * (j + 1)]
                    )
                mv = work.tile([128, 2], f32, tag="mv")
                nc.vector.bn_aggr(mv[:sw], stats[:sw])
                std = work.tile([128, 1], f32, tag="std")
                nc.scalar.activation(
                    std[:sw], mv[:sw, 1:2], AF.Sqrt, bias=eps_t[:sw]
                )
                rstd = work.tile([128, 1], f32, tag="rstd")
                nc.vector.reciprocal(rstd[:sw], std[:sw])
                xn = work.tile([128, D], bf16, tag="xn")
                nc.vector.tensor_scalar(
                    xn[:sw], xt, scalar1=mv[:sw, 0:1], scalar2=rstd[:sw],
                    op0=OP.subtract, op1=OP.mult,
                )
                for (k0, kn) in TGRP:
                    pt = psT.tile([128, 512], bf16, tag="pst")
                    for kt in range(k0, k0 + kn):
                        kp = KP[kt]
                        nc.tensor.transpose(
                            pt[:kp, 128 * (kt - k0) : 128 * (kt - k0) + sw],
                            xn[:sw, 128 * kt : 128 * kt + kp],
                            ident[:sw, :sw],
                        )
                    kpg = KP[k0]  # 128 for full groups, 56 for the tail
                    if kn == 1 or sw == 128:
                        nc.scalar.activation(
                            actT[:kpg, k0 : k0 + kn, s0 : s0 + sw],
                            pt[:kpg, : 128 * (kn - 1) + sw],
                            AF.Identity,
                        )
                    else:
                        for kt in range(k0, k0 + kn):
                            nc.scalar.activation(
                                actT[: KP[kt], kt, s0 : s0 + sw],
                                pt[: KP[kt], 128 * (kt - k0) : 128 * (kt - k0) + sw],
                                AF.Identity,
                            )

        # ================= P1: LN1 =================
        layernorm_transpose()
        for st in range(N_ST):
            nc.vector.tensor_tensor(
                r1[: SW[st], st, :], r1[: SW[st], st, :], bo_bc[: SW[st]], OP.add
            )

        # ============ P2 + P3 + P4 (attention, scoped pools) ============
        pVcm = tc.tile_pool(name="pV", bufs=1)
        pV = pVcm.__enter__()
        v_sb = pV.tile([128, N_ST, H, DK + 7], bf16, tag="vsb", name="v_sb")
        nc.gpsimd.memset(v_sb, 1.0)

        # ---- P2: V (token-major, +ones col) ----
        with tc.tile_pool(name="pP2w", bufs=1) as pP2w:
            wv_t = pP2w.tile([128, N_KT, D], bf16, tag="wv")
            nc.sync.dma_start(wv_t, wv_d)
            for vb in range(N_VB):
                c0 = VBW * vb
                for st in range(N_ST):
                    sw = SW[st]
                    pv = psA.tile([128, 2, 512], f32, tag="pa")
                    for kt in range(N_KT):
                        kp = KP[kt]
                        nc.tensor.matmul(
                            pv[:sw, 0, :VBW],
                            actT[:kp, kt, 128 * st : 128 * st + sw],
                            wv_t[:kp, kt, c0 : c0 + VBW],
                            start=(kt == 0),
                            stop=(kt == N_KT - 1),
                        )
                    for hh in range(4):
                        h = 4 * vb + hh
                        nc.vector.tensor_tensor(
                            v_sb[:sw, st, h, :DK],
                            pv[:sw, 0, DK * hh : DK * hh + DK],
                            bv_bc[:sw, DK * h : DK * h + DK],
                            OP.add,
                        )

        # ---- P3: attention ----
        pATcm = tc.tile_pool(name="pAT", bufs=1)
        pAT = pATcm.__enter__()
        aT = pAT.tile([DK, H, S], bf16, tag="aT", name="aT")
        with tc.tile_pool(name="pP3c", bufs=1) as pP3c, \
             tc.tile_pool(name="pP3", bufs=2) as pP3, \
             tc.tile_pool(name="pP3e", bufs=4) as pP3e:
            rl_s = pP3c.tile([DK, DK], bf16, tag="rl")
            nc.sync.dma_start(rl_s, rl_d)
            cosT = pP3c.tile([DK, S], bf16, tag="cosT")
            nc.sync.dma_start(cosT, cost_d)
            sinT = pP3c.tile([DK, S], bf16, tag="sinT")
            nc.sync.dma_start(sinT, sint_d)
            cm_s = pP3c.tile([128, 4, 512], bf16, tag="cmask")
            nc.sync.dma_start(cm_s, cmask_d.rearrange("t p f -> p t f"))
            bqk_s = pP3c.tile([DK, 2, H], f32, tag="bqk")
            nc.sync.dma_start(bqk_s, bqk_d)
            bq_s = bqk_s[:, 0, :]
            bk_s = bqk_s[:, 1, :]

            for h in range(H):
                wqk_t = pP3.tile([128, N_KT, 2, DK], bf16, tag="wqk")
                nc.sync.dma_start(wqk_t, wqk_d[h])

                qT = pP3.tile([DK, S], bf16, tag="qT")
                kT = pP3.tile([DK, S], bf16, tag="kT")
                for (wi, b_s, outT) in ((0, bq_s, qT), (1, bk_s, kT)):
                    for (q0, qw) in QCH:
                        pq = psT.tile([128, 512], f32, tag="pst")
                        for kt in range(N_KT):
                            kp = KP[kt]
                            nc.tensor.matmul(
                                pq[:DK, :qw],
                                wqk_t[:kp, kt, wi, :],
                                actT[:kp, kt, q0 : q0 + qw],
                                start=(kt == 0),
                                stop=(kt == N_KT - 1),
                            )
                        raw = pP3.tile([DK, 512], bf16, tag="qraw")
                        nc.scalar.activation(
                            raw[:, :qw], pq[:DK, :qw], AF.Identity,
                            bias=b_s[:, h : h + 1],
                        )
                        prot = psT.tile([128, 512], f32, tag="pst")
                        nc.tensor.matmul(
                            prot[:DK, :qw], rl_s, raw[:, :qw], start=True, stop=True
                        )
                        t1 = pP3.tile([DK, 512], bf16, tag="ropet1")
                        nc.vector.tensor_tensor(
                            t1[:, :qw], raw[:, :qw], cosT[:, q0 : q0 + qw], OP.mult
                        )
                        t2 = pP3.tile([DK, 512], bf16, tag="ropet2")
                        nc.vector.tensor_tensor(
                            t2[:, :qw], prot[:DK, :qw], sinT[:, q0 : q0 + qw],
                            OP.mult,
                        )
                        nc.vector.tensor_tensor(
                            outT[:, q0 : q0 + qw], t1[:, :qw], t2[:, :qw], OP.add
                        )

                for (q0, qw) in QCH:
                    kmax = min(N_ST, (q0 + qw + 127) // 128)
                    pat = psB.tile([128, 512], f32, tag="pb")
                    for i0 in range(0, kmax, 2):
                        kn = min(2, kmax - i0)
                        pe2 = psA.tile([128, 2, 512], f32, tag="pa")
                        for j in range(kn):
                            i = i0 + j
                            ksz = SW[i]
                            nc.tensor.matmul(
                                pe2[:ksz, j, :qw],
                                kT[:, 128 * i : 128 * i + ksz],
                                qT[:, q0 : q0 + qw],
                                start=True,
                                stop=True,
                            )
                        et = pP3e.tile([128, 2, 512], bf16, tag="et")
                        ks0 = SW[i0]
                        if kn == 2 and SW[i0 + 1] == ks0:
                            nc.scalar.activation(
                                et[:ks0, :, :qw], pe2[:ks0, :, :qw], AF.Exp,
                                scale=SCALE,
                            )
                        else:
                            for j in range(kn):
                                nc.scalar.activation(
                                    et[: SW[i0 + j], j, :qw],
                                    pe2[: SW[i0 + j], j, :qw],
                                    AF.Exp,
                                    scale=SCALE,
                                )
                        for j in range(kn):
                            i = i0 + j
                            ksz = SW[i]
                            if 128 * i + ksz - 1 > q0:
                                t_ = i - q0 // 128
                                nc.vector.tensor_tensor(
                                    et[:ksz, j, :qw], et[:ksz, j, :qw],
                                    cm_s[:ksz, t_, :qw], OP.mult,
                                )
                            nc.tensor.matmul(
                                pat[: DK + 7, :qw],
                                v_sb[:ksz, i, h, :],
                                et[:ksz, j, :qw],
                                start=(i == 0),
                                stop=(i == kmax - 1),
                            )
                    rec = pP3.tile([1, 512], bf16, tag="rec")
                    with nc.allow_low_precision(reason="bf16 denom bcast"):
                        nc.vector.reciprocal(rec[:, :qw], pat[DK + 6 : DK + 7, :qw])
                    bc = pP3.tile([DK, 512], bf16, tag="bc")
                    nc.gpsimd.partition_broadcast(bc[:, :qw], rec[:, :qw])
                    nc.vector.tensor_tensor(
                        aT[:, h, q0 : q0 + qw], pat[:DK, :qw], bc[:, :qw], OP.mult
                    )

        # ---- P4: out-proj token-major + residual ----
        with tc.tile_pool(name="pP4", bufs=1) as pP4:
            wo_s = pP4.tile([DK, H, D], bf16, tag="wo")
            nc.sync.dma_start(wo_s, wo_d)
            for st in range(N_ST):
                sw = SW[st]
                s0 = 128 * st
                for vb in range(N_VB):
                    c0 = VBW * vb
                    po = psA.tile([128, 2, 512], f32, tag="pa")
                    for h in range(H):
                        nc.tensor.matmul(
                            po[:sw, 0, :VBW],
                            aT[:, h, s0 : s0 + sw],
                            wo_s[:, h, c0 : c0 + VBW],
                            start=(h == 0),
                            stop=(h == H - 1),
                        )
                    nc.vector.tensor_tensor(
                        r1[:sw, st, c0 : c0 + VBW], r1[:sw, st, c0 : c0 + VBW],
                        po[:sw, 0, :VBW], OP.add,
                    )

        pATcm.__exit__(None, None, None)
        pVcm.__exit__(None, None, None)

        # ================= P5: LN2 =================
        layernorm_transpose()
        for st in range(N_ST):
            nc.vector.tensor_tensor(
                r1[: SW[st], st, :], r1[: SW[st], st, :], b2_bc[: SW[st]], OP.add
            )

        # ================= P6: FFN in 3 S-chunks ==========================
        with tc.tile_pool(name="pW2", bufs=1) as pW2, \
             tc.tile_pool(name="pF", bufs=2) as pF, \
             tc.tile_pool(name="pFh", bufs=1) as pFh:
            w2_s = pW2.tile([128, N_FT, D], bf16, tag="w2")
            nc.sync.dma_start(w2_s, w2_d)
            b1_s = col_param(pW2, "b1", b1_d, N_FT, FSZ)
            b3_s = col_param(pW2, "b3", b3_d, N_FT, FSZ)
            Ht = pFh.tile([128, N_FT, 512], bf16, tag="Ht", name="Ht")
            for (g0, gw) in QCH:
                for ft in range(N_FT):
                    fsz = FSZ[ft]
                    w13_t = pF.tile([128, N_KT, 2, 128], bf16, tag="w13")
                    nc.sync.dma_start(w13_t, w13_d[ft])
                    p1_ = psA.tile([128, 2, 512], f32, tag="pa")
                    p3_ = psB.tile([128, 512], f32, tag="pb")
                    for kt in range(N_KT):
                        kp = KP[kt]
                        nc.tensor.matmul(
                            p1_[:fsz, 0, :gw], w13_t[:kp, kt, 0, :fsz],
                            actT[:kp, kt, g0 : g0 + gw],
                            start=(kt == 0), stop=(kt == N_KT - 1),
                        )
                        nc.tensor.matmul(
                            p3_[:fsz, :gw], w13_t[:kp, kt, 1, :fsz],
                            actT[:kp, kt, g0 : g0 + gw],
                            start=(kt == 0), stop=(kt == N_KT - 1),
                        )
                    h1s = pF.tile([128, 512], bf16, tag="h1s")
                    nc.scalar.activation(
                        h1s[:fsz, :gw], p1_[:fsz, 0, :gw], AF.Silu,
                        bias=b1_s[:fsz, ft : ft + 1],
                    )
                    h3b = pF.tile([128, 512], bf16, tag="h3b")
                    nc.scalar.activation(
                        h3b[:fsz, :gw], p3_[:fsz, :gw], AF.Identity,
                        bias=b3_s[:fsz, ft : ft + 1],
                    )
                    nc.vector.tensor_tensor(
                        Ht[:fsz, ft, :gw], h1s[:fsz, :gw], h3b[:fsz, :gw], OP.mult
                    )
                for j in range((gw + 127) // 128):
                    st = g0 // 128 + j
                    sw = SW[st]
                    for vb in range(N_VB):
                        c0 = VBW * vb
                        pf = psA.tile([128, 2, 512], f32, tag="pa")
                        for ft in range(N_FT):
                            fsz = FSZ[ft]
                            nc.tensor.matmul(
                                pf[:sw, 0, :VBW],
                                Ht[:fsz, ft, 128 * j : 128 * j + sw],
                                w2_s[:fsz, ft, c0 : c0 + VBW],
                                start=(ft == 0), stop=(ft == N_FT - 1),
                            )
                        nc.vector.tensor_tensor(
                            r1[:sw, st, c0 : c0 + VBW],
                            r1[:sw, st, c0 : c0 + VBW],
                            pf[:sw, 0, :VBW], OP.add,
                        )

        # ================= P7: store =================
        nc.sync.dma_start(
            out_d[: 128 * (N_ST - 1), :].rearrange("(o p) d -> p o d", p=128),
            r1[:, : N_ST - 1, :],
        )
        nc.sync.dma_start(out_d[128 * (N_ST - 1) :, :], r1[: SW[-1], N_ST - 1, :])

    nc.compile()
    return nc


def _host_inputs(inputs):
    """Shared (per-core-identical) input map pieces, from full inputs."""
    import ml_dtypes

    f = lambda k: np.asarray(inputs[k], np.float32)

    def to_bf16(a):
        return np.ascontiguousarray(np.asarray(a, np.float32)).astype(
            ml_dtypes.bfloat16
        )

    g1 = f("ln1_g"); be1 = f("ln1_b"); g2 = f("ln2_g"); be2 = f("ln2_b")
    Wq = f("Wq") * g1[:, None]
    Wk = f("Wk") * g1[:, None]
    Wv = f("Wv") * g1[:, None]
    Wo = f("Wo")
    W1 = f("W1") * g2[:, None]
    W3 = f("W3") * g2[:, None]
    W2 = f("W2")
    bq = f("bq") + be1 @ f("Wq")
    bk = f("bk") + be1 @ f("Wk")
    bv = f("bv") + be1 @ f("Wv")
    b1 = f("b1") + be2 @ f("W1")
    b3 = f("b3") + be2 @ f("W3")
    bo = f("bo")
    b2 = f("b2")

    cos = np.ascontiguousarray(f("rope_cos").T)   # [DK, S]
    sin = np.ascontiguousarray(f("rope_sin").T)
    rl = np.zeros((DK, DK), np.float32)
    hdk = DK // 2
    rl[np.arange(hdk) + hdk, np.arange(hdk)] = -1.0
    rl[np.arange(hdk), np.arange(hdk) + hdk] = 1.0
    ident = np.eye(128, dtype=np.float32)

    def pad_rows(w, rows):
        out = np.zeros((rows, w.shape[1]), np.float32)
        out[: w.shape[0]] = w
        return out

    KR = N_KT * 128
    # [H, 128, N_KT, DK]: (h, p, o, d) = Wq[o*128+p, h*90+d]
    wqr = pad_rows(Wq, KR).reshape(N_KT, 128, H, DK).transpose(2, 1, 0, 3)
    wkr = pad_rows(Wk, KR).reshape(N_KT, 128, H, DK).transpose(2, 1, 0, 3)
    # [128, N_KT, D]
    wvr = pad_rows(Wv, KR).reshape(N_KT, 128, D).transpose(1, 0, 2)
    # [DK, H, D]: (p, h, c) = Wo[h*90+p, c]
    wor = Wo.reshape(H, DK, D).transpose(1, 0, 2)
    # [N_FT, 128, N_KT, 128]: (ft, p, o, m) = W1[o*128+p, ft*128+m]
    FR = N_FT * 128
    w1p = np.zeros((KR, FR), np.float32); w1p[:D, :FF] = W1
    w3p = np.zeros((KR, FR), np.float32); w3p[:D, :FF] = W3
    w1r = w1p.reshape(N_KT, 128, N_FT, 128).transpose(2, 1, 0, 3)
    w3r = w3p.reshape(N_KT, 128, N_FT, 128).transpose(2, 1, 0, 3)
    # [128, N_FT, D]: (p, ft, c) = W2[ft*128+p, c]
    w2p = np.zeros((FR, D), np.float32); w2p[:FF] = W2
    w2r = w2p.reshape(N_FT, 128, D).transpose(1, 0, 2)

    cmask = np.zeros((4, 128, 512), np.float32)
    for t in range(4):
        p_, f_ = np.mgrid[0:128, 0:512]
        cmask[t] = (f_ >= p_ + 128 * t).astype(np.float32)
    bvb = np.broadcast_to(bv[None, :], (128, D)).copy()
    bob = np.broadcast_to(bo[None, :], (128, D)).copy()
    b2b = np.broadcast_to(b2[None, :], (128, D)).copy()
    wqk = np.stack([wqr, wkr], axis=3)           # [H, 128, N_KT, 2, DK]
    w13 = np.stack([w1r, w3r], axis=3)           # [N_FT, 128, N_KT, 2, 128]
    bqk = np.stack([bq.reshape(H, DK).T, bk.reshape(H, DK).T], axis=1)
    ball = np.stack([bvb, bob, b2b], axis=1)     # [128, 3, D]
    return {
        "wqkr": to_bf16(wqk), "wvr": to_bf16(wvr),
        "wor": to_bf16(wor), "w13r": to_bf16(w13), "w2r": to_bf16(w2r),
        "bqk": np.ascontiguousarray(bqk),
        "ball": np.ascontiguousarray(ball),
        "b1": b1, "b3": b3,
        "cost": to_bf16(cos), "sint": to_bf16(sin), "rl": to_bf16(rl),
        "ident": to_bf16(ident), "cmask": to_bf16(cmask),
    }


def kernel(**inputs):
    from concourse.bass_utils import run_bass_kernel_spmd

    if "nc" not in _CACHE:
        _CACHE["nc"] = _build()
    nc = _CACHE["nc"]

    shared = _host_inputs(inputs)
    x = np.asarray(inputs["x"], np.float32)
    in_maps = [dict(shared, x=np.ascontiguousarray(x[b])) for b in range(B)]
    res = run_bass_kernel_spmd(nc, in_maps, list(range(B))).results
    out = np.stack([res[b]["out"] for b in range(B)], axis=0)
    return out.astype(np.float32)


# revision 11
# speedup vs baseline: 1.7692x; 1.0126x over previous
"""Trainium2 Bass kernel for a pre-norm transformer encoder layer with RoPE,
causal attention and SwiGLU FFN.

Sharding: data-parallel over batch (B=8 -> 8 NeuronCores, one batch element
per core).  Each core runs the full layer on its [S=1300, D=1080] slice.

v2 design (vs the DRAM-scratch baseline):
  - everything stays in SBUF: V, attn^T and the FFN hidden tensor are never
    staged to DRAM scratch.
  - all matmul operands are bf16 (psum fp32): halves SBUF footprint, halves
    weight DMA traffic, enables FWL weight loads and 4x DVE modes.
  - LN gamma/beta are folded into the projection weights host-side, so the
    LN transpose evacuation is a plain (batched) copy.
  - softmax denominator comes for free from a ones-column appended to V
    (attn^T matmul computes numerator rows 0..89 and the denominator row 90).
  - out-proj and FFN-W2 accumulate token-major straight into the fp32
    residual r1 (no PE transposes / accumulate-DMAs on the output path).

Per-core dataflow:
  P1  r1 = x (token-major, fp32); LN1 stats (bn_stats) -> xn (bf16),
      PE-transpose -> actT [128, 9, 1300] bf16; r1 += bo broadcast
  P2  V = x2 @ Wv per 360-col block -> v_sb [128, 11, 12, 91] bf16
      (col 90 memset to 1.0 = denominator ones-column)
  P3  per head: Q/K proj (psum) + bias, RoPE (rotation matmul + DVE),
      per q-chunk: scores K^T Q (psum pairs), exp (scalar, scale folded),
      causal mask on diagonal tiles, attnT = [V|1]^T E accumulated in psum,
      normalize via reciprocal + ones-row broadcast matmul -> aT bf16
  P4  out-proj token-major: po[s,360] = sum_h aT_h^T Wo_h; r1 += po
  P5  LN2 -> actT (overwrite); r1 += b2 broadcast
  P6  FFN in 3 S-chunks: Ht = silu(x2'W1+b1)*(x2'W3+b3) bf16;
      token-major W2: pf[s,360] = sum_ft Ht_ft^T W2_ft; r1 += pf
  P7  out = r1 (two DMAs)
"""

import sys

sys.path.insert(0, "/opt/trn_rl_repo")

import math

import numpy as np

B, S, D, H, DK, FF = 8, 1300, 1080, 12, 90, 3240
EPS = 1e-5

N_ST = (S + 127) // 128                      # 11 token tiles
SW = [128] * (N_ST - 1) + [S - 128 * (N_ST - 1)]   # last = 20
N_KT = (D + 127) // 128                      # 9
KP = [128] * (N_KT - 1) + [D - 128 * (N_KT - 1)]   # last = 56
QCH = [(0, 512), (512, 512), (1024, 276)]    # q/s chunks (128-aligned starts)
N_FT = (FF + 127) // 128                     # 26
FSZ = [128] * (N_FT - 1) + [FF - 128 * (N_FT - 1)]  # last = 40
N_VB = 3
VBW = D // N_VB                              # 360
TGRP = [(0, 4), (4, 4), (8, 1)]              # k-tile groups for LN evac

_CACHE = {}


def _build():
    from contextlib import ExitStack

    import concourse.bacc as bacc
    import concourse.mybir as mybir
    import concourse.tile as tile

    f32 = mybir.dt.float32
    bf16 = mybir.dt.bfloat16
    AF = mybir.ActivationFunctionType
    OP = mybir.AluOpType

    nc = bacc.Bacc("TRN2", target_bir_lowering=False, debug=False)

    def din(name, shape, dt=f32):
        return nc.dram_tensor(name, shape, dt, kind="ExternalInput").ap()

    def dout(name, shape, dt=f32):
        return nc.dram_tensor(name, shape, dt, kind="ExternalOutput").ap()

    x_d = din("x", (S, D))
    wqk_d = din("wqkr", (H, 128, N_KT, 2, DK), bf16)
    wv_d = din("wvr", (128, N_KT, D), bf16)
    wo_d = din("wor", (DK, H, D), bf16)
    w13_d = din("w13r", (N_FT, 128, N_KT, 2, 128), bf16)
    w2_d = din("w2r", (128, N_FT, D), bf16)
    bqk_d = din("bqk", (DK, 2, H))
    ball_d = din("ball", (128, 3, D))
    b1_d = din("b1", (FF,))
    b3_d = din("b3", (FF,))
    cost_d = din("cost", (DK, S), bf16)
    sint_d = din("sint", (DK, S), bf16)
    rl_d = din("rl", (DK, DK), bf16)
    ident_d = din("ident", (128, 128), bf16)
    cmask_d = din("cmask", (4, 128, 512), bf16)

    out_d = dout("out", (S, D))

    SCALE = 1.0 / math.sqrt(DK)

    with tile.TileContext(nc) as tc, ExitStack() as ctx:
        glob = ctx.enter_context(tc.tile_pool(name="glob", bufs=1))
        work = ctx.enter_context(tc.tile_pool(name="work", bufs=3))
        psA = ctx.enter_context(tc.tile_pool(name="psA", bufs=2, space="PSUM"))
        psB = ctx.enter_context(tc.tile_pool(name="psB", bufs=2, space="PSUM"))
        psT = ctx.enter_context(tc.tile_pool(name="psT", bufs=2, space="PSUM"))

        # ---------- persistent tensors ----------
        actT = glob.tile([128, N_KT, S], bf16, tag="actT")    # x2T / x2'T
        r1 = glob.tile([128, N_ST, D], f32, tag="r1")         # residual

        # ---------- load x into r1 first (token-major; gates LN1) ----------
        for st in range(N_ST):
            nc.sync.dma_start(
                r1[: SW[st], st, :], x_d[128 * st : 128 * st + SW[st], :]
            )

        # ---------- persistent constants (ACT HWDGE ring) ----------
        ident = glob.tile([128, 128], bf16, tag="ident")
        nc.scalar.dma_start(ident, ident_d)
        eps_t = glob.tile([128, 1], f32, tag="eps")
        nc.vector.memset(eps_t, EPS)
        ball = glob.tile([128, 3, D], f32, tag="ball")
        nc.scalar.dma_start(ball, ball_d)
        bv_bc = ball[:, 0, :]
        bo_bc = ball[:, 1, :]
        b2_bc = ball[:, 2, :]

        def col_param(pool, name, dram, n, psz):
            t = pool.tile([128, n], f32, tag=name, name=name)
            full = sum(1 for p in psz if p == 128)
            if full:
                nc.scalar.dma_start(
                    t[:, :full],
                    dram[0 : 128 * full].rearrange("(o p) -> p o", p=128),
                )
            for i in range(full, n):
                o = sum(psz[:i])
                nc.scalar.dma_start(t[: psz[i], i : i + 1], dram[o : o + psz[i], None])
            return t

        # ---------- helper: LN + transpose into actT (gamma/beta folded into
        # the downstream weights, so the evacuation is a plain copy) ----------
        def layernorm_transpose():
            for st in range(N_ST):
                sw = SW[st]
                s0 = 128 * st
                xt = r1[:sw, st, :]
                stats = work.tile([128, 3, 6], f32, tag="stats")
                for j in range(3):
                    nc.vector.bn_stats(
                        stats[:sw, j, :], xt[:, VBW * j : VBW * (j + 1)]
                    )
                mv = work.tile([128, 2], f32, tag="mv")
                nc.vector.bn_aggr(mv[:sw], stats[:sw])
                std = work.tile([128, 1], f32, tag="std")
                nc.scalar.activation(
                    std[:sw], mv[:sw, 1:2], AF.Sqrt, bias=eps_t[:sw]
                )
                rstd = work.tile([128, 1], f32, tag="rstd")
                nc.vector.reciprocal(rstd[:sw], std[:sw])
                xn = work.tile([128, D], bf16, tag="xn")
                nc.vector.tensor_scalar(
                    xn[:sw], xt, scalar1=mv[:sw, 0:1], scalar2=rstd[:sw],
                    op0=OP.subtract, op1=OP.mult,
                )
                for (k0, kn) in TGRP:
                    pt = psT.tile([128, 512], bf16, tag="pst")
                    for kt in range(k0, k0 + kn):
                        kp = KP[kt]
                        nc.tensor.transpose(
                            pt[:kp, 128 * (kt - k0) : 128 * (kt - k0) + sw],
                            xn[:sw, 128 * kt : 128 * kt + kp],
                            ident[:sw, :sw],
                        )
                    kpg = KP[k0]  # 128 for full groups, 56 for the tail
                    if kn == 1 or sw == 128:
                        nc.scalar.activation(
                            actT[:kpg, k0 : k0 + kn, s0 : s0 + sw],
                            pt[:kpg, : 128 * (kn - 1) + sw],
                            AF.Identity,
                        )
                    else:
                        for kt in range(k0, k0 + kn):
                            nc.scalar.activation(
                                actT[: KP[kt], kt, s0 : s0 + sw],
                                pt[: KP[kt], 128 * (kt - k0) : 128 * (kt - k0) + sw],
                                AF.Identity,
                            )

        # ================= P1: LN1 =================
        layernorm_transpose()
        for st in range(N_ST):
            nc.vector.tensor_tensor(
                r1[: SW[st], st, :], r1[: SW[st], st, :], bo_bc[: SW[st]], OP.add
            )

        # ============ P2 + P3 + P4 (attention, scoped pools) ============
        pVcm = tc.tile_pool(name="pV", bufs=1)
        pV = pVcm.__enter__()
        v_sb = pV.tile([128, N_ST, H, DK + 7], bf16, tag="vsb", name="v_sb")
        nc.gpsimd.memset(v_sb, 1.0)

        # ---- P2: V (token-major, +ones col) ----
        with tc.tile_pool(name="pP2w", bufs=1) as pP2w:
            wv_t = pP2w.tile([128, N_KT, D], bf16, tag="wv")
            nc.scalar.dma_start(wv_t, wv_d)
            for vb in range(N_VB):
                c0 = VBW * vb
                for st in range(N_ST):
                    sw = SW[st]
                    pv = psA.tile([128, 2, 512], f32, tag="pa")
                    for kt in range(N_KT):
                        kp = KP[kt]
                        nc.tensor.matmul(
                            pv[:sw, 0, :VBW],
                            actT[:kp, kt, 128 * st : 128 * st + sw],
                            wv_t[:kp, kt, c0 : c0 + VBW],
                            start=(kt == 0),
                            stop=(kt == N_KT - 1),
                        )
                    for hh in range(4):
                        h = 4 * vb + hh
                        nc.vector.tensor_tensor(
                            v_sb[:sw, st, h, :DK],
                            pv[:sw, 0, DK * hh : DK * hh + DK],
                            bv_bc[:sw, DK * h : DK * h + DK],
                            OP.add,
                        )

        # ---- P3: attention ----
        pATcm = tc.tile_pool(name="pAT", bufs=1)
        pAT = pATcm.__enter__()
        aT = pAT.tile([DK, H, S], bf16, tag="aT", name="aT")
        with tc.tile_pool(name="pP3c", bufs=1) as pP3c, \
             tc.tile_pool(name="pP3", bufs=2) as pP3, \
             tc.tile_pool(name="pP3e", bufs=4) as pP3e:
            rl_s = pP3c.tile([DK, DK], bf16, tag="rl")
            nc.scalar.dma_start(rl_s, rl_d)
            cosT = pP3c.tile([DK, S], bf16, tag="cosT")
            nc.scalar.dma_start(cosT, cost_d)
            sinT = pP3c.tile([DK, S], bf16, tag="sinT")
            nc.scalar.dma_start(sinT, sint_d)
            cm_s = pP3c.tile([128, 4, 512], bf16, tag="cmask")
            nc.scalar.dma_start(cm_s, cmask_d.rearrange("t p f -> p t f"))
            bqk_s = pP3c.tile([DK, 2, H], f32, tag="bqk")
            nc.scalar.dma_start(bqk_s, bqk_d)
            bq_s = bqk_s[:, 0, :]
            bk_s = bqk_s[:, 1, :]

            for h in range(H):
                wqk_t = pP3.tile([128, N_KT, 2, DK], bf16, tag="wqk")
                nc.sync.dma_start(wqk_t, wqk_d[h])

                qT = pP3.tile([DK, S], bf16, tag="qT")
                kT = pP3.tile([DK, S], bf16, tag="kT")
                for (wi, b_s, outT) in ((0, bq_s, qT), (1, bk_s, kT)):
                    for (q0, qw) in QCH:
                        pq = psT.tile([128, 512], f32, tag="pst")
                        for kt in range(N_KT):
                            kp = KP[kt]
                            nc.tensor.matmul(
                                pq[:DK, :qw],
                                wqk_t[:kp, kt, wi, :],
                                actT[:kp, kt, q0 : q0 + qw],
                                start=(kt == 0),
                                stop=(kt == N_KT - 1),
                            )
                        raw = pP3.tile([DK, 512], bf16, tag="qraw")
                        nc.scalar.activation(
                            raw[:, :qw], pq[:DK, :qw], AF.Identity,
                            bias=b_s[:, h : h + 1],
                        )
                        prot = psT.tile([128, 512], f32, tag="pst")
                        nc.tensor.matmul(
                            prot[:DK, :qw], rl_s, raw[:, :qw], start=True, stop=True
                        )
                        t1 = pP3.tile([DK, 512], bf16, tag="ropet1")
                        nc.vector.tensor_tensor(
                            t1[:, :qw], raw[:, :qw], cosT[:, q0 : q0 + qw], OP.mult
                        )
                        t2 = pP3.tile([DK, 512], bf16, tag="ropet2")
                        nc.vector.tensor_tensor(
                            t2[:, :qw], prot[:DK, :qw], sinT[:, q0 : q0 + qw],
                            OP.mult,
                        )
                        nc.vector.tensor_tensor(
                            outT[:, q0 : q0 + qw], t1[:, :qw], t2[:, :qw], OP.add
                        )

                for (q0, qw) in QCH:
                    kmax = min(N_ST, (q0 + qw + 127) // 128)
                    pat = psB.tile([128, 512], f32, tag="pb")
                    for i0 in range(0, kmax, 2):
                        kn = min(2, kmax - i0)
                        pe2 = psA.tile([128, 2, 512], f32, tag="pa")
                        for j in range(kn):
                            i = i0 + j
                            ksz = SW[i]
                            nc.tensor.matmul(
                                pe2[:ksz, j, :qw],
                                kT[:, 128 * i : 128 * i + ksz],
                                qT[:, q0 : q0 + qw],
                                start=True,
                                stop=True,
                            )
                        et = pP3e.tile([128, 2, 512], bf16, tag="et")
                        ks0 = SW[i0]
                        if kn == 2 and SW[i0 + 1] == ks0:
                            nc.scalar.activation(
                                et[:ks0, :, :qw], pe2[:ks0, :, :qw], AF.Exp,
                                scale=SCALE,
                            )
                        else:
                            for j in range(kn):
                                nc.scalar.activation(
                                    et[: SW[i0 + j], j, :qw],
                                    pe2[: SW[i0 + j], j, :qw],
                                    AF.Exp,
                                    scale=SCALE,
                                )
                        for j in range(kn):
                            i = i0 + j
                            ksz = SW[i]
                            if 128 * i + ksz - 1 > q0:
                                t_ = i - q0 // 128
                                nc.vector.tensor_tensor(
                                    et[:ksz, j, :qw], et[:ksz, j, :qw],
                                    cm_s[:ksz, t_, :qw], OP.mult,
                                )
                            nc.tensor.matmul(
                                pat[: DK + 7, :qw],
                                v_sb[:ksz, i, h, :],
                                et[:ksz, j, :qw],
                                start=(i == 0),
                                stop=(i == kmax - 1),
                            )
                    rec = pP3.tile([1, 512], bf16, tag="rec")
                    with nc.allow_low_precision(reason="bf16 denom bcast"):
                        nc.vector.reciprocal(rec[:, :qw], pat[DK + 6 : DK + 7, :qw])
                    bc = pP3.tile([DK, 512], bf16, tag="bc")
                    nc.gpsimd.partition_broadcast(bc[:, :qw], rec[:, :qw])
                    nc.vector.tensor_tensor(
                        aT[:, h, q0 : q0 + qw], pat[:DK, :qw], bc[:, :qw], OP.mult
                    )

        # ---- P4: out-proj token-major + residual ----
        with tc.tile_pool(name="pP4", bufs=1) as pP4:
            wo_s = pP4.tile([DK, H, D], bf16, tag="wo")
            nc.scalar.dma_start(wo_s, wo_d)
            for st in range(N_ST):
                sw = SW[st]
                s0 = 128 * st
                for vb in range(N_VB):
                    c0 = VBW * vb
                    po = psA.tile([128, 2, 512], f32, tag="pa")
                    for h in range(H):
                        nc.tensor.matmul(
                            po[:sw, 0, :VBW],
                            aT[:, h, s0 : s0 + sw],
                            wo_s[:, h, c0 : c0 + VBW],
                            start=(h == 0),
                            stop=(h == H - 1),
                        )
                    nc.vector.tensor_tensor(
                        r1[:sw, st, c0 : c0 + VBW], r1[:sw, st, c0 : c0 + VBW],
                        po[:sw, 0, :VBW], OP.add,
                    )

        pATcm.__exit__(None, None, None)
        pVcm.__exit__(None, None, None)

        # ================= P5: LN2 =================
        layernorm_transpose()
        for st in range(N_ST):
            nc.vector.tensor_tensor(
                r1[: SW[st], st, :], r1[: SW[st], st, :], b2_bc[: SW[st]], OP.add
            )

        # ================= P6: FFN in 3 S-chunks ==========================
        with tc.tile_pool(name="pW2", bufs=1) as pW2, \
             tc.tile_pool(name="pF", bufs=2) as pF, \
             tc.tile_pool(name="pFh", bufs=1) as pFh:
            w2_s = pW2.tile([128, N_FT, D], bf16, tag="w2")
            nc.scalar.dma_start(w2_s, w2_d)
            b1_s = col_param(pW2, "b1", b1_d, N_FT, FSZ)
            b3_s = col_param(pW2, "b3", b3_d, N_FT, FSZ)
            Ht = pFh.tile([128, N_FT, 512], bf16, tag="Ht", name="Ht")
            for (g0, gw) in QCH:
                for ft in range(N_FT):
                    fsz = FSZ[ft]
                    w13_t = pF.tile([128, N_KT, 2, 128], bf16, tag="w13")
                    nc.sync.dma_start(w13_t, w13_d[ft])
                    p1_ = psA.tile([128, 2, 512], f32, tag="pa")
                    p3_ = psB.tile([128, 512], f32, tag="pb")
                    for kt in range(N_KT):
                        kp = KP[kt]
                        nc.tensor.matmul(
                            p1_[:fsz, 0, :gw], w13_t[:kp, kt, 0, :fsz],
                            actT[:kp, kt, g0 : g0 + gw],
                            start=(kt == 0), stop=(kt == N_KT - 1),
                        )
                        nc.tensor.matmul(
                            p3_[:fsz, :gw], w13_t[:kp, kt, 1, :fsz],
                            actT[:kp, kt, g0 : g0 + gw],
                            start=(kt == 0), stop=(kt == N_KT - 1),
                        )
                    h1s = pF.tile([128, 512], bf16, tag="h1s")
                    nc.scalar.activation(
                        h1s[:fsz, :gw], p1_[:fsz, 0, :gw], AF.Silu,
                        bias=b1_s[:fsz, ft : ft + 1],
                    )
                    h3b = pF.tile([128, 512], bf16, tag="h3b")
                    nc.scalar.activation(
                        h3b[:fsz, :gw], p3_[:fsz, :gw], AF.Identity,
                        bias=b3_s[:fsz, ft : ft + 1],
                    )
                    nc.vector.tensor_tensor(
                        Ht[:fsz, ft, :gw], h1s[:fsz, :gw], h3b[:fsz, :gw], OP.mult
                    )
                for j in range((gw + 127) // 128):
                    st = g0 // 128 + j
                    sw = SW[st]
                    for vb in range(N_VB):
                        c0 = VBW * vb
                        pf = psA.tile([128, 2, 512], f32, tag="pa")
                        for ft in range(N_FT):
                            fsz = FSZ[ft]
                            nc.tensor.matmul(
                                pf[:sw, 0, :VBW],
                                Ht[:fsz, ft, 128 * j : 128 * j + sw],
                                w2_s[:fsz, ft, c0 : c0 + VBW],
                                start=(ft == 0), stop=(ft == N_FT - 1),
                            )
                        nc.vector.tensor_tensor(
                            r1[:sw, st, c0 : c0 + VBW],
                            r1[:sw, st, c0 : c0 + VBW],
                            pf[:sw, 0, :VBW], OP.add,
                        )

        # ================= P7: store =================
        nc.sync.dma_start(
            out_d[: 128 * (N_ST - 1), :].rearrange("(o p) d -> p o d", p=128),
            r1[:, : N_ST - 1, :],
        )
        nc.sync.dma_start(out_d[128 * (N_ST - 1) :, :], r1[: SW[-1], N_ST - 1, :])

    nc.compile()
    return nc


def _host_inputs(inputs):
    """Shared (per-core-identical) input map pieces, from full inputs."""
    import ml_dtypes

    f = lambda k: np.asarray(inputs[k], np.float32)

    def to_bf16(a):
        return np.ascontiguousarray(np.asarray(a, np.float32)).astype(
            ml_dtypes.bfloat16
        )

    g1 = f("ln1_g"); be1 = f("ln1_b"); g2 = f("ln2_g"); be2 = f("ln2_b")
    Wq = f("Wq") * g1[:, None]
    Wk = f("Wk") * g1[:, None]
    Wv = f("Wv") * g1[:, None]
    Wo = f("Wo")
    W1 = f("W1") * g2[:, None]
    W3 = f("W3") * g2[:, None]
    W2 = f("W2")
    bq = f("bq") + be1 @ f("Wq")
    bk = f("bk") + be1 @ f("Wk")
    bv = f("bv") + be1 @ f("Wv")
    b1 = f("b1") + be2 @ f("W1")
    b3 = f("b3") + be2 @ f("W3")
    bo = f("bo")
    b2 = f("b2")

    cos = np.ascontiguousarray(f("rope_cos").T)   # [DK, S]
    sin = np.ascontiguousarray(f("rope_sin").T)
    rl = np.zeros((DK, DK), np.float32)
    hdk = DK // 2
    rl[np.arange(hdk) + hdk, np.arange(hdk)] = -1.0
    rl[np.arange(hdk), np.arange(hdk) + hdk] = 1.0
    ident = np.eye(128, dtype=np.float32)

    def pad_rows(w, rows):
        out = np.zeros((rows, w.shape[1]), np.float32)
        out[: w.shape[0]] = w
        return out

    KR = N_KT * 128
    # [H, 128, N_KT, DK]: (h, p, o, d) = Wq[o*128+p, h*90+d]
    wqr = pad_rows(Wq, KR).reshape(N_KT, 128, H, DK).transpose(2, 1, 0, 3)
    wkr = pad_rows(Wk, KR).reshape(N_KT, 128, H, DK).transpose(2, 1, 0, 3)
    # [128, N_KT, D]
    wvr = pad_rows(Wv, KR).reshape(N_KT, 128, D).transpose(1, 0, 2)
    # [DK, H, D]: (p, h, c) = Wo[h*90+p, c]
    wor = Wo.reshape(H, DK, D).transpose(1, 0, 2)
    # [N_FT, 128, N_KT, 128]: (ft, p, o, m) = W1[o*128+p, ft*128+m]
    FR = N_FT * 128
    w1p = np.zeros((KR, FR), np.float32); w1p[:D, :FF] = W1
    w3p = np.zeros((KR, FR), np.float32); w3p[:D, :FF] = W3
    w1r = w1p.reshape(N_KT, 128, N_FT, 128).transpose(2, 1, 0, 3)
    w3r = w3p.reshape(N_KT, 128, N_FT, 128).transpose(2, 1, 0, 3)
    # [128, N_FT, D]: (p, ft, c) = W2[ft*128+p, c]
    w2p = np.zeros((FR, D), np.float32); w2p[:FF] = W2
    w2r = w2p.reshape(N_FT, 128, D).transpose(1, 0, 2)

    cmask = np.zeros((4, 128, 512), np.float32)
    for t in range(4):
        p_, f_ = np.mgrid[0:128, 0:512]
        cmask[t] = (f_ >= p_ + 128 * t).astype(np.float32)
    bvb = np.broadcast_to(bv[None, :], (128, D)).copy()
    bob = np.broadcast_to(bo[None, :], (128, D)).copy()
    b2b = np.broadcast_to(b2[None, :], (128, D)).copy()
    wqk = np.stack([wqr, wkr], axis=3)           # [H, 128, N_KT, 2, DK]
    w13 = np.stack([w1r, w3r], axis=3)           # [N_FT, 128, N_KT, 2, 128]
    bqk = np.stack([bq.reshape(H, DK).T, bk.reshape(H, DK).T], axis=1)
    ball = np.stack([bvb, bob, b2b], axis=1)     # [128, 3, D]
    return {
        "wqkr": to_bf16(wqk), "wvr": to_bf16(wvr),
        "wor": to_bf16(wor), "w13r": to_bf16(w13), "w2r": to_bf16(w2r),
        "bqk": np.ascontiguousarray(bqk),
        "ball": np.ascontiguousarray(ball),
        "b1": b1, "b3": b3,
        "cost": to_bf16(cos), "sint": to_bf16(sin), "rl": to_bf16(rl),
        "ident": to_bf16(ident), "cmask": to_bf16(cmask),
    }


def kernel(**inputs):
    from concourse.bass_utils import run_bass_kernel_spmd

    if "nc" not in _CACHE:
        _CACHE["nc"] = _build()
    nc = _CACHE["nc"]

    shared = _host_inputs(inputs)
    x = np.asarray(inputs["x"], np.float32)
    in_maps = [dict(shared, x=np.ascontiguousarray(x[b])) for b in range(B)]
    res = run_bass_kernel_spmd(nc, in_maps, list(range(B))).results
    out = np.stack([res[b]["out"] for b in range(B)], axis=0)
    return out.astype(np.float32)
